# revision 50
# baseline (speedup 1.0000x reference)
"""BasicTransformerBlock on 8 TRN2 NeuronCores.

Sharding: sequence-parallel, zero collectives. The [B=2, N=2048, D=512]
residual stream is split into 8 row-blocks of 512 (4 cores per batch
element). Every core recomputes the cheap batch-wide work it needs
(adaln1 + K/V projections over its batch's 2048 rows, cond K/V), and does
attention / FFN only for its own 512 query rows.

Host-side prep (part of kernel()'s sharding layer, shared across cores):
weights are pre-cast to bf16 and pre-rearranged into [partition, ktile,
cols] SBUF layouts and cond is pre-transposed, so every DMA is a fast
contiguous transfer (no software-DGE casting scatter loads) and no on-chip
cond transposes are needed. x stays f32 (layernorm stats need it). Per-core
x is pre-rotated with np.roll so "own" rows are always rows 0:512;
attention is permutation-invariant over keys, so rolled K/V is fine.

Schedule highlights (engine-level):
 - Attention uses transposed scores sT[j, i] so exp() runs on ScalarE
   straight out of PSUM; the 65th v-column of ones makes the softmax
   denominator fall out of the attn@v matmul. Score matmuls are DENSE
   (K=128): both heads of a pair stacked in the stationary operand, query
   operand zero-padded per head. The PE_HAM activity monitor only grants
   the 2.4 GHz clock to full-array matmuls; K=64 scores (even row-tile-
   paired ones) run throttled at 1.2 GHz.
 - The exp stream is the kernel's bottleneck (ScalarE ACTIVATE is
   (N+352)/1.2 ns, ~100us for the 12.6M softmax elements per core), so
   attention is built as one flat software-pipelined granule loop across
   all 4 head pairs: exp(g-1) / scores(g) / attn@v(g-2), with score
   granules alternating 3/2 PSUM slots between two tiles (5 banks) + 2
   accumulator banks + 1 filler bank. Whole-tile rotation expresses the
   2-deep pipeline exactly, so ScalarE runs back-to-back exps while the PE
   fills its slack with the cross-attention K/V projection chains
   (fillers) and softmax tails.
 - adaln: bn_stats chunks -> rstd/nmr -> xn on ScalarE -> PE transpose ->
   fused (1+scale)/shift PSUM drain on DVE, interleaved per 512-row block
   with the dense QKV projection matmuls (PSUM->SBUF drains ride on
   ScalarE Copy where DVE is the busier engine).
 - FFN streams per-ut: zu/zg chains -> Gelu(ScalarE) -> GEGLU stt(DVE) ->
   4 persistent PSUM y-accumulators, output DMA overlapped per row-block.
"""

import contextlib

import ml_dtypes
import numpy as np

import concourse.bass as bass
import concourse.mybir as mybir
import concourse.tile as tile
from concourse import bacc
from concourse.bass_utils import run_bass_kernel_spmd
from concourse.masks import make_identity

dt = mybir.dt
AF = mybir.ActivationFunctionType
OP = mybir.AluOpType

B, N, D = 2, 2048, 512
NCTX = 1024          # cond length
H = 8                # heads
HD = D // H          # 64
EPS = 1e-5
P = 128              # partitions
NCORES = 8
ROWS = 512           # own rows per core
NB = N               # batch rows per core (2048)
SCALE = HD ** -0.5   # 0.125

f32 = dt.float32
bf16 = dt.bfloat16

_CACHED = {}


def _adaln_stats(nc, stat_pool, src_tiles, n_tiles, eps_sb, chunk=4):
    """bn_stats/aggr + rstd/nmr for n_tiles row-tiles. Returns (rstd_all, nmr_all)."""
    mv_all = stat_pool.tile([P, n_tiles, 2], f32)
    rstd_all = stat_pool.tile([P, n_tiles], f32)
    nmr_all = stat_pool.tile([P, n_tiles], f32)
    for c0 in range(0, n_tiles, chunk):
        for it in range(c0, c0 + chunk):
            stats = stat_pool.tile([P, 6], f32, tag="stats")
            nc.vector.bn_stats(stats, src_tiles(it))
            nc.vector.bn_aggr(mv_all[:, it, :], stats)
        cs = slice(c0, c0 + chunk)
        nc.scalar.activation(rstd_all[:, cs], mv_all[:, cs, 1], AF.Sqrt,
                             bias=eps_sb, scale=1.0)
        nc.vector.reciprocal(rstd_all[:, cs], rstd_all[:, cs])
        nc.vector.scalar_tensor_tensor(
            nmr_all[:, cs], mv_all[:, cs, 0], -1.0, rstd_all[:, cs],
            op0=OP.mult, op1=OP.mult,
        )
    return rstd_all, nmr_all


def _adaln_apply_tile(nc, xn_pool, pst_pool, src, it, ab, rstd_all, nmr_all,
                      hT, ident_bf16, on_act=False):
    """One tile: xn = (x-mean)*rstd -> PE transpose -> (1+scale)/shift -> hT."""
    xn = xn_pool.tile([P, 512], bf16, tag="xn")
    if on_act:
        nc.scalar.activation(xn, src, AF.Identity,
                             bias=nmr_all[:, it:it + 1],
                             scale=rstd_all[:, it:it + 1])
    else:
        nc.vector.tensor_scalar(
            xn, src, rstd_all[:, it:it + 1], nmr_all[:, it:it + 1],
            op0=OP.mult, op1=OP.add,
        )
    xnt = pst_pool.tile([P, 4, P], bf16, tag="xnt")
    for b in range(4):
        nc.tensor.transpose(xnt[:, b, :], xn[:, b * P:(b + 1) * P], ident_bf16)
    for b in range(4):
        nc.vector.tensor_scalar(
            hT[:, b, it * P:(it + 1) * P], xnt[:, b, :],
            ab[:, b:b + 1], ab[:, 4 + b:5 + b],
            op0=OP.mult, op1=OP.add,
        )
    return xn


def _adaln_to_hT(nc, tc, src_tiles, n_tiles, ab, hT, ident_bf16, eps_sb, name):
    with contextlib.ExitStack() as actx:
        stat_pool = actx.enter_context(tc.tile_pool(name=f"{name}_stat", bufs=4))
        xn_pool = actx.enter_context(tc.tile_pool(name=f"{name}_xn", bufs=3))
        pst_pool = actx.enter_context(
            tc.tile_pool(name=f"{name}_pst", bufs=2, space="PSUM"))
        warm_pool = actx.enter_context(
            tc.tile_pool(name=f"{name}_warm", bufs=1, space="PSUM"))
        wps = warm_pool.tile([P, P], f32, name=f"{name}_wps")
        rstd, nmr = _adaln_stats(nc, stat_pool, src_tiles, n_tiles, eps_sb,
                                 chunk=1)
        for it in range(n_tiles):
            xn = _adaln_apply_tile(nc, xn_pool, pst_pool, src_tiles(it), it,
                                   ab, rstd, nmr, hT, ident_bf16, on_act=True)
            # dense warm-keeper matmuls chained on this tile's xn: PE
            # transposes don't register as HAM activity, so without these
            # the whole gap chain runs at 1.2 GHz
            for _ in range(2):
                nc.tensor.matmul(wps, xn[:, 0:P], ident_bf16,
                                 start=True, stop=True)


def _attention(nc, tc, act, qpad, kT, v, njt, wo, ob_row, ones_row,
               x_res, x_out, name, fillers=None):
    """Dense-score attention for 8 heads (4 pairs) over own 512 rows.

    qpad: [128, 4, 2, ROWS] bf16 zero-padded per head half. Dense K=128
    score matmuls keep the PE_HAM activity monitor granting 2.4 GHz;
    row-tile-paired K=64 scores measure ~35us more throttle.
    kT:   [128, 4, njt*128] bf16 (partitions = paired head dims).
    v:    [128, njt, 8, 65] bf16 (col 64 of each head = 1.0).
    Writes x_out = attn_out @ wo + ob + x_res  (all [128, 4, 512] f32).

    Granules alternate 3/2 score slots between two PSUM tiles (5 banks),
    leaving one bank for `fillers`: {step: fn(pool)} closures that emit
    independent dense matmul chains into the PE's slack (the exp stream is
    the bottleneck), e.g. the cross-attention K/V projections.
    """
    av_all = act.tile([P, 4, ROWS], bf16, tag="tH")
    S = 2 * njt                       # score slots per pair
    # alternating 3/2-slot granules; G kept EVEN so the two tile tags
    # alternate seamlessly across pair boundaries
    sizes = [3, 2] * 6 + [1, 1] if njt == 16 else [3, 2, 3, 2, 2, 2, 1, 1]
    assert sum(sizes) == S and len(sizes) % 2 == 0
    offs = [0]
    for t in sizes:
        offs.append(offs[-1] + t)
    G = len(sizes)                    # granules per pair
    M = 4 * G                         # global granules across the 4 pairs
    with (
        tc.tile_pool(name=f"{name}_ps_s", bufs=1, space="PSUM") as ps_s,
        tc.tile_pool(name=f"{name}_ps_av", bufs=1, space="PSUM") as ps_av,
        tc.tile_pool(name=f"{name}_ps_f", bufs=1, space="PSUM") as ps_f,
        tc.tile_pool(name=f"{name}_et", bufs=3) as et_pool,
        tc.tile_pool(name=f"{name}_dn", bufs=4) as dn_pool,
        tc.tile_pool(name=f"{name}_rb", bufs=2) as rb_pool,
    ):
        def pair_tail(pht, pavp, rb_ps):
            # softmax denominators: row 64 of each accumulator. Broadcast
            # across partitions with K=1 matmuls into slot 0 of the NEXT
            # score granule's tile (its slot-0 score matmul then overwrites
            # after the reciprocal reads -- read-before-write deps keep it
            # correct), then one fast reciprocal for the pair.
            for hl in range(2):
                dnm = dn_pool.tile([1, ROWS], bf16, tag="dnm")
                nc.vector.tensor_copy(dnm, pavp[HD:HD + 1, hl, :])
                nc.tensor.matmul(
                    rb_ps[hl * HD:(hl + 1) * HD, :],
                    ones_row[0:1, 0:HD], dnm,
                    start=True, stop=True,
                )
            rb_sb = rb_pool.tile([P, ROWS], f32, tag="rb")
            nc.vector.reciprocal_approx_fast(rb_sb, rb_ps)
            for hl in range(2):
                po = hl * HD
                nc.vector.scalar_tensor_tensor(
                    av_all[po:po + HD, pht, :],
                    pavp[0:HD, hl, :], 1.0, rb_sb[po:po + HD, :],
                    op0=OP.mult, op1=OP.mult,
                )

        def lmap(m):
            p, g = divmod(m, G)
            return p, offs[g], offs[g + 1]

        def sg_alloc(m):
            if m % 2 == 0:
                return ps_s.tile([P, 3, ROWS], f32, tag="sgr3", name="sg3")
            return ps_s.tile([P, 2, ROWS], f32, tag="sgr2", name="sg2")

        ets = {}
        sgr = {}
        avps = {}
        pend_sg = None
        for m in range(M + 2):
            if 1 <= m <= M:
                p, lo, hi = lmap(m - 1)
                ng = hi - lo
                et = et_pool.tile([P, 3, ROWS], bf16, tag="et")
                ets[m - 1] = (et, p, lo, hi)
                nc.scalar.activation(
                    et[:, 0:ng, :], sgr[m - 1][:, 0:ng, :], AF.Exp,
                    scale=SCALE,
                )
            if m < M:
                p, lo, hi = lmap(m)
                from_pend = pend_sg is not None
                if from_pend:
                    sg = pend_sg
                    pend_sg = None
                else:
                    sg = sg_alloc(m)
                sgr[m] = sg
                # slot 0 of a pend tile carries the previous pair's
                # denominator broadcast: write it LAST so its WAW-after-
                # reciprocal wait overlaps the other slots' matmuls
                order = reversed(range(lo, hi)) if from_pend else range(lo, hi)
                for s in order:
                    jt, hl = s // 2, s % 2
                    nc.tensor.matmul(
                        sg[:, s - lo, :],
                        kT[:, p, jt * P:(jt + 1) * P],
                        qpad[:, p, hl, :],
                        start=True, stop=True,
                    )
            if m >= 2:
                et, p, lo, hi = ets.pop(m - 2)
                sgr.pop(m - 2, None)
                if p not in avps:
                    avps[p] = ps_av.tile([P, 2, ROWS], f32, tag="av",
                                         name=f"avp{p}")
                for i, s in enumerate(range(lo, hi)):
                    jt, hl = s // 2, s % 2
                    nc.tensor.matmul(
                        avps[p][0:HD + 1, hl, :], v[:, jt, 2 * p + hl, :],
                        et[:, i, :],
                        start=(jt == 0), stop=(jt == njt - 1),
                    )
                if hi == S:  # pair p's accumulators complete: emit its tail
                    pend_sg = sg_alloc(m + 1)   # granule consumed next step
                    pair_tail(p, avps[p], pend_sg[:, 0, :])
            if fillers and m in fillers:
                fillers[m](ps_f)
    # out-projection + bias + residual
    with tc.tile_pool(name=f"{name}_ps_o", bufs=2, space="PSUM") as ps_o:
        for it in range(4):
            ps = ps_o.tile([P, D], f32, tag="o")
            for dt_ in range(4):
                nc.tensor.matmul(
                    ps, av_all[:, dt_, it * P:(it + 1) * P], wo[:, dt_, :],
                    start=(dt_ == 0), stop=False,
                )
            nc.tensor.matmul(
                ps, ones_row[0:1, 0:P], ob_row, start=False, stop=True,
            )
            nc.vector.tensor_tensor(x_out[:, it, :], ps, x_res[:, it, :], op=OP.add)


def build():
    nc = bacc.Bacc(None, target_bir_lowering=False)

    # -------- I/O (weights arrive pre-cast/pre-arranged from the host) ----
    xb = nc.dram_tensor("xb", [NB, D], f32, kind="ExternalInput")
    condT_d = nc.dram_tensor("condT", [P, 4, NCTX], bf16, kind="ExternalInput")
    tT_d = nc.dram_tensor("tT", [P, 4], bf16, kind="ExternalInput")
    nw_d = {}
    nb_d = {}
    for l in (1, 2, 4):
        nw_d[l] = nc.dram_tensor(f"nw{l}", [P, 4, 2 * D], bf16,
                                 kind="ExternalInput")
        nb_d[l] = nc.dram_tensor(f"nb{l}", [2 * D], f32, kind="ExternalInput")
    as_d = {}
    aob_d = {}
    for a in (1, 2):
        as_d[a] = nc.dram_tensor(f"a{a}s", [P, 4, 4, D], bf16,
                                 kind="ExternalInput")
        aob_d[a] = nc.dram_tensor(f"a{a}ob", [D], bf16, kind="ExternalInput")
    ffw1_d = nc.dram_tensor("ffw1", [P, 4, 8 * D], bf16, kind="ExternalInput")
    ffw2_d = nc.dram_tensor("ffw2", [P, 16, D], bf16, kind="ExternalInput")
    ffb1_d = nc.dram_tensor("ffb1", [P, 32], f32, kind="ExternalInput")
    ffb2_d = nc.dram_tensor("ffb2", [D], bf16, kind="ExternalInput")
    out = nc.dram_tensor("out", [ROWS, D], f32, kind="ExternalOutput")

    with tile.TileContext(nc) as tc, contextlib.ExitStack() as ctx:
        const = ctx.enter_context(tc.tile_pool(name="const", bufs=1))
        wpool = ctx.enter_context(tc.tile_pool(name="wpool", bufs=1))
        act = ctx.enter_context(tc.tile_pool(name="act", bufs=1))
        xr_pool = ctx.enter_context(tc.tile_pool(name="xrp", bufs=6))
        n1_stat = ctx.enter_context(tc.tile_pool(name="n1_stat", bufs=4))

        ident_bf16 = const.tile([P, P], bf16)
        make_identity(nc, ident_bf16)
        ident_f32 = const.tile([P, P], f32)
        make_identity(nc, ident_f32)
        ones_row = const.tile([1, P], bf16)
        nc.vector.memset(ones_row, 1.0)
        eps_sb = const.tile([P, 1], f32)
        nc.vector.memset(eps_sb, EPS)

        # ---------------- DMA issue order --------------------------------
        # qACT: weights in first-use order.  qSP: x, small rows, ff tail.
        tT = const.tile([P, 4], bf16)
        nc.scalar.dma_start(tT, tT_d[:])
        ab = {}
        with (
            tc.tile_pool(name="nwp", bufs=1) as nwp,
            tc.tile_pool(name="embp", bufs=1) as embp,
        ):
            nw_sb = {}
            for l in (1, 2):
                nw_sb[l] = nwp.tile([P, 4, 2 * D], bf16, tag=f"nw{l}",
                                    name=f"nw_sb{l}")
                nc.scalar.dma_start(nw_sb[l], nw_d[l][:])

            a_sb = {}
            stacks = {}
            for a, wtag in ((1, "wbig1"), (2, "wbig2")):
                stack = wpool.tile([P, 4, 4, D], bf16, tag=wtag,
                                   name=f"a{a}stk")
                stacks[a] = stack
                for wi, w in enumerate("qkvo"):
                    a_sb[a, w] = stack[:, :, wi, :]
            pass
            for a in (1, 2):
                ob = wpool.tile([1, D], bf16, tag=f"a{a}ob", name=f"a{a}ob_sb")
                a_sb[a, "ob"] = ob

            h1T = act.tile([P, 4, NB], bf16, tag="tA")
            own_x = act.tile([P, 4, D], f32, tag="tE")
            x_tiles = {}
            for it in range(16):
                if it < 4:
                    dst = own_x[:, it, :]
                else:
                    dst = xr_pool.tile([P, D], f32, tag="xr", name=f"xr{it}")
                nc.gpsimd.dma_start(dst, xb[:][it * P:(it + 1) * P, :])
                x_tiles[it] = dst

            # condT + k2T share the tX region
            ctk2 = act.tile([P, 2, 4, NCTX], bf16, tag="tX")
            condT = ctk2[:, 0, :, :]
            k2T = ctk2[:, 1, :, :]
            # big weights + condT on the otherwise-idle SWDGE queue: HWDGE
            # trigger instructions cost ~2.5us EACH on their engine's queue
            # and were starving the ACT stats chain.
            nc.gpsimd.dma_start(stacks[1], as_d[1][:])
            nc.gpsimd.dma_start(a_sb[1, "ob"],
                                aob_d[1][:].rearrange("(a n) -> a n", a=1))
            nc.gpsimd.dma_start(stacks[2], as_d[2][:])
            nc.gpsimd.dma_start(a_sb[2, "ob"],
                                aob_d[2][:].rearrange("(a n) -> a n", a=1))
            nc.gpsimd.dma_start(condT, condT_d[:])
            nw_sb[4] = nwp.tile([P, 4, 2 * D], bf16, tag="nw1", name="nw_sb4")
            nc.gpsimd.dma_start(nw_sb[4], nw_d[4][:])

            nb_row = {}
            for l in (1, 2, 4):
                nb_row[l] = embp.tile([1, 2 * D], f32, tag="nbrow",
                                      name=f"nb_row{l}")
                nc.sync.dma_start(nb_row[l],
                                  nb_d[l][:].rearrange("(a n) -> a n", a=1))
            b1_sb = const.tile([P, 32], f32)
            nc.sync.dma_start(b1_sb, ffb1_d[:])
            b2_row = const.tile([1, D], bf16)
            nc.sync.dma_start(b2_row, ffb2_d[:].rearrange("(a n) -> a n", a=1))
            # ff weights on qSP after x: landed long before the FFN needs
            # them, but the buffers alias a1s/a2s so they wait on attention.
            w1_sb = wpool.tile([P, 4, 8 * D], bf16, tag="wbig1")
            nc.gpsimd.dma_start(w1_sb, ffw1_d[:])
            w2_sb = wpool.tile([P, 16, D], bf16, tag="wbig2")
            nc.gpsimd.dma_start(w2_sb, ffw2_d[:])

            # PE warmup: dependency-free matmuls fill the startup DMA window
            with tc.tile_pool(name="warm", bufs=1, space="PSUM") as warm_pool:
                wps = warm_pool.tile([P, P], f32)
                for _ in range(30):
                    nc.tensor.matmul(wps, ident_bf16, ident_bf16,
                                     start=True, stop=True)

            # ------------- norm scale/shift params -----------------------
            # only ab[1] gates the adaln1 loop; l=2,4 run after it so their
            # nw DMA arrival never stalls the PE stream.
            def emb_ab(l, ps_emb):
                emb_ps = ps_emb.tile([1, 2 * D], f32, tag="embps",
                                     name=f"emb_ps{l}")
                for half in range(2):
                    for kt in range(4):
                        nc.tensor.matmul(
                            emb_ps[:, half * D:(half + 1) * D],
                            tT[:, kt:kt + 1],
                            nw_sb[l][:, kt, half * D:(half + 1) * D],
                            start=(kt == 0), stop=(kt == 3),
                        )
                emb_row = embp.tile([1, 2 * D], f32, tag="embrow",
                                    name=f"emb_row{l}")
                nc.vector.tensor_tensor(emb_row, emb_ps, nb_row[l], op=OP.add)
                ab_l = const.tile([P, 8], f32, tag=f"ab{l}", name=f"ab_{l}")
                for col in range(8):
                    tp = ps_emb.tile([P, 1], f32, tag="embT")
                    nc.tensor.transpose(
                        tp, emb_row[0:1, col * P:(col + 1) * P],
                        ident_f32[0:1, 0:1]
                    )
                    nc.vector.tensor_scalar(
                        ab_l[:, col:col + 1], tp,
                        1.0 if col < 4 else 0.0, None, op0=OP.add,
                    )
                ab[l] = ab_l

            with tc.tile_pool(name="ps_emb1", bufs=2, space="PSUM") as pe1:
                emb_ab(1, pe1)

            # --------- adaln1 apply + projections, interleaved -----------
            # Per 512-row block: stats chunk -> xn (ACT) -> PE transpose ->
            # affine (DVE) -> dense k1/v1/q matmuls, with the PSUM->SBUF
            # drains on the otherwise-idle ACT engine.
            k1T = act.tile([P, 4, NB], bf16, tag="tB")
            v1 = act.tile([P, 16, H, HD + 1], bf16, tag="tC")
            q1pad = act.tile([P, 4, 2, ROWS], bf16, tag="tD")
            nc.gpsimd.memset(v1[:, :, :, HD:HD + 1], 1.0)
            nc.gpsimd.memset(q1pad, 0.0)
            mv1 = n1_stat.tile([P, 16, 2], f32)
            rstd1 = n1_stat.tile([P, 16], f32)
            nmr1 = n1_stat.tile([P, 16], f32)
            v2 = act.tile([P, 8, H, HD + 1], bf16, tag="tI")
            nc.gpsimd.memset(v2[:, :, :, HD:HD + 1], 1.0)
            with (
                tc.tile_pool(name="n1_xn", bufs=3) as xn_pool,
                tc.tile_pool(name="n1_pst", bufs=2, space="PSUM") as pst_pool,
                tc.tile_pool(name="ps_proj1", bufs=4, space="PSUM") as ps_proj,
            ):
                for jc in range(4):
                    cs = slice(4 * jc, 4 * jc + 4)
                    for it in range(4 * jc, 4 * jc + 4):
                        stats = n1_stat.tile([P, 6], f32, tag="stats")
                        nc.vector.bn_stats(stats, x_tiles[it])
                        nc.vector.bn_aggr(mv1[:, it, :], stats)
                    nc.scalar.activation(rstd1[:, cs], mv1[:, cs, 1], AF.Sqrt,
                                         bias=eps_sb, scale=1.0)
                    nc.vector.reciprocal(rstd1[:, cs], rstd1[:, cs])
                    nc.vector.scalar_tensor_tensor(
                        nmr1[:, cs], mv1[:, cs, 0], -1.0, rstd1[:, cs],
                        op0=OP.mult, op1=OP.mult,
                    )
                    for it in range(4 * jc, 4 * jc + 4):
                        _adaln_apply_tile(nc, xn_pool, pst_pool, x_tiles[it],
                                          it, ab[1], rstd1, nmr1, h1T,
                                          ident_bf16, on_act=True)
                    for dt_ in range(4):
                        ps = ps_proj.tile([P, 512], f32, tag="proj")
                        for kt in range(4):
                            nc.tensor.matmul(
                                ps,
                                a_sb[1, "k"][:, kt, dt_ * P:(dt_ + 1) * P],
                                h1T[:, kt, jc * 512:(jc + 1) * 512],
                                start=(kt == 0), stop=(kt == 3),
                            )
                        nc.scalar.activation(
                            k1T[:, dt_, jc * 512:(jc + 1) * 512], ps, AF.Copy
                        )
                    for jt in range(4 * jc, 4 * jc + 4):
                        ps = ps_proj.tile([P, 512], f32, tag="proj")
                        for kt in range(4):
                            nc.tensor.matmul(
                                ps,
                                h1T[:, kt, jt * P:(jt + 1) * P],
                                a_sb[1, "v"][:, kt, :],
                                start=(kt == 0), stop=(kt == 3),
                            )
                        nc.scalar.activation(
                            v1[:, jt, :, 0:HD],
                            ps.rearrange("p (h d) -> p h d", h=H), AF.Copy
                        )
                    if jc == 0:
                        for dt_ in range(4):
                            ps = ps_proj.tile([P, 512], f32, tag="proj")
                            for kt in range(4):
                                nc.tensor.matmul(
                                    ps,
                                    a_sb[1, "q"][:, kt, dt_ * P:(dt_ + 1) * P],
                                    h1T[:, kt, 0:ROWS],
                                    start=(kt == 0), stop=(kt == 3),
                                )
                            nc.scalar.activation(q1pad[0:HD, dt_, 0, :],
                                                 ps[0:HD, :], AF.Copy)
                            nc.scalar.activation(q1pad[HD:P, dt_, 1, :],
                                                 ps[HD:P, :], AF.Copy)

            with tc.tile_pool(name="ps_emb2", bufs=2, space="PSUM") as pe2:
                emb_ab(2, pe2)
                emb_ab(4, pe2)

        # ---------------- attention 1 ------------------------------------
        # cross-attn K/V projections ride along as fillers in attn1's PE
        # slack (the exp stream is the bottleneck there); each chain uses
        # the one spare PSUM bank.
        def mk_k2(dt_, cjc):
            def f(pool):
                ps = pool.tile([P, 512], f32, tag="fps", name="fps")
                for kt in range(4):
                    nc.tensor.matmul(
                        ps,
                        a_sb[2, "k"][:, kt, dt_ * P:(dt_ + 1) * P],
                        condT[:, kt, cjc * 512:(cjc + 1) * 512],
                        start=(kt == 0), stop=(kt == 3),
                    )
                nc.vector.tensor_copy(
                    k2T[:, dt_, cjc * 512:(cjc + 1) * 512], ps
                )
            return f

        def mk_v2(jt):
            def f(pool):
                ps = pool.tile([P, 512], f32, tag="fps", name="fps")
                for kt in range(4):
                    nc.tensor.matmul(
                        ps,
                        condT[:, kt, jt * P:(jt + 1) * P],
                        a_sb[2, "v"][:, kt, :],
                        start=(kt == 0), stop=(kt == 3),
                    )
                nc.vector.tensor_copy(
                    v2[:, jt, :, 0:HD], ps.rearrange("p (h d) -> p h d", h=H)
                )
            return f

        fill1 = {}
        fns = ([mk_k2(dt_, cjc) for dt_ in range(4) for cjc in range(2)]
               + [mk_v2(jt) for jt in range(8)])
        for i, fn in enumerate(fns):
            fill1[20 + 2 * i] = fn

        x2 = act.tile([P, 4, D], f32, tag="tF")
        _attention(nc, tc, act, q1pad, k1T, v1, 16, a_sb[1, "o"],
                   a_sb[1, "ob"], ones_row, own_x, x2, "att1",
                   fillers=fill1)

        # ---------------- adaln2 + cross-attn ----------------------------
        h2T = act.tile([P, 4, ROWS], bf16, tag="tH")
        _adaln_to_hT(nc, tc, lambda it: x2[:, it, :], 4, ab[2], h2T,
                     ident_bf16, eps_sb, "n2")

        q2pad = act.tile([P, 4, 2, ROWS], bf16, tag="tD")
        nc.gpsimd.memset(q2pad, 0.0)
        with tc.tile_pool(name="ps_proj2b", bufs=2, space="PSUM") as ps_proj:
            for dt_ in range(4):
                ps = ps_proj.tile([P, 512], f32, tag="proj")
                for kt in range(4):
                    nc.tensor.matmul(
                        ps,
                        a_sb[2, "q"][:, kt, dt_ * P:(dt_ + 1) * P],
                        h2T[:, kt, :],
                        start=(kt == 0), stop=(kt == 3),
                    )
                nc.scalar.activation(q2pad[0:HD, dt_, 0, :], ps[0:HD, :],
                                     AF.Copy)
                nc.scalar.activation(q2pad[HD:P, dt_, 1, :], ps[HD:P, :],
                                     AF.Copy)

        x3 = act.tile([P, 4, D], f32, tag="tG")
        _attention(nc, tc, act, q2pad, k2T, v2, 8, a_sb[2, "o"],
                   a_sb[2, "ob"], ones_row, x2, x3, "att2")

        # ---------------- adaln3 + GEGLU FFN -----------------------------
        h3T = act.tile([P, 4, ROWS], bf16, tag="tJ")
        _adaln_to_hT(nc, tc, lambda it: x3[:, it, :], 4, ab[4], h3T,
                     ident_bf16, eps_sb, "n4")

        # per-ut pipeline: zu/zg -> gelu/stt -> 4 y-accumulator matmuls.
        # y accumulates in 4 persistent PSUM banks across all 16 ut chunks.
        ugT = act.tile([P, 16, ROWS], bf16, tag="tA")
        out_sb = act.tile([P, 4, D], f32, tag="tC")
        with (
            tc.tile_pool(name="ps_z", bufs=4, space="PSUM") as ps_z,
            tc.tile_pool(name="ps_y", bufs=1, space="PSUM") as ps_y,
            tc.tile_pool(name="gact", bufs=3) as gact_pool,
        ):
            y_ps = ps_y.tile([P, 4, D], f32)
            for ut in range(16):
                zu = ps_z.tile([P, ROWS], f32, tag="z")
                zg = ps_z.tile([P, ROWS], f32, tag="z")
                for kt in range(4):
                    nc.tensor.matmul(
                        zu, w1_sb[:, kt, ut * P:(ut + 1) * P],
                        h3T[:, kt, :], start=(kt == 0), stop=(kt == 3),
                    )
                for kt in range(4):
                    nc.tensor.matmul(
                        zg, w1_sb[:, kt, (16 + ut) * P:(17 + ut) * P],
                        h3T[:, kt, :], start=(kt == 0), stop=(kt == 3),
                    )
                gact = gact_pool.tile([P, ROWS], bf16, tag="gact")
                nc.scalar.activation(
                    gact, zg, AF.Gelu, bias=b1_sb[:, 16 + ut:17 + ut], scale=1.0
                )
                nc.vector.scalar_tensor_tensor(
                    ugT[:, ut, :], zu, b1_sb[:, ut:ut + 1], gact,
                    op0=OP.add, op1=OP.mult,
                )
                for it in range(4):
                    nc.tensor.matmul(
                        y_ps[:, it, :], ugT[:, ut, it * P:(it + 1) * P],
                        w2_sb[:, ut, :],
                        start=(ut == 0), stop=False,
                    )
            for it in range(4):
                nc.tensor.matmul(
                    y_ps[:, it, :], ones_row[0:1, 0:P], b2_row,
                    start=False, stop=True,
                )
                nc.vector.tensor_tensor(
                    out_sb[:, it, :], y_ps[:, it, :], x3[:, it, :], op=OP.add
                )
                nc.sync.dma_start(out[:][it * P:(it + 1) * P, :],
                                  out_sb[:, it, :])

    nc.compile()
    return nc


def _prep_shared(inputs):
    """Pre-cast weights to bf16 and pre-arrange into SBUF layouts (host-side
    layout prep, shared by all cores)."""
    bf = ml_dtypes.bfloat16

    def pkn(w, ktiles):
        # [ktiles*128, n] f32 -> [128, ktiles, n] bf16
        n = w.shape[1]
        return np.ascontiguousarray(
            w.reshape(ktiles, P, n).transpose(1, 0, 2).astype(bf))

    shared = {}
    for l in (1, 2, 4):
        shared[f"nw{l}"] = pkn(np.asarray(inputs[f"n{l}_w"], np.float32), 4)
        shared[f"nb{l}"] = np.ascontiguousarray(inputs[f"n{l}_b"], np.float32)
    for a in (1, 2):
        ws = [pkn(np.asarray(inputs[f"a{a}_{w}"], np.float32), 4)
              for w in "qkvo"]
        shared[f"a{a}s"] = np.ascontiguousarray(np.stack(ws, axis=2))
        shared[f"a{a}ob"] = np.asarray(inputs[f"a{a}_ob"], np.float32).astype(bf)
    shared["ffw1"] = pkn(np.asarray(inputs["ff_w1"], np.float32), 4)
    shared["ffw2"] = pkn(np.asarray(inputs["ff_w2"], np.float32), 16)
    shared["ffb1"] = np.ascontiguousarray(
        np.asarray(inputs["ff_b1"], np.float32).reshape(32, P).T)
    shared["ffb2"] = np.asarray(inputs["ff_b2"], np.float32).astype(bf)
    return shared


def _shard_inputs(inputs):
    """Build the 8 per-core input maps."""
    bf = ml_dtypes.bfloat16
    x = np.ascontiguousarray(inputs["x"], dtype=np.float32)
    t = np.ascontiguousarray(inputs["t"], dtype=np.float32)
    cond = np.ascontiguousarray(inputs["cond"], dtype=np.float32)
    shared = _prep_shared(inputs)
    per_batch = {}
    for b in range(B):
        condT = cond[b].T.reshape(4, P, NCTX).transpose(1, 0, 2)
        tT = t[b, 0].reshape(4, P).T
        per_batch[b] = (
            np.ascontiguousarray(condT.astype(bf)),
            np.ascontiguousarray(tT.astype(bf)),
        )
    in_maps = []
    for c in range(NCORES):
        b = c // 4
        r0 = (c % 4) * ROWS
        m = dict(shared)
        m["xb"] = np.ascontiguousarray(np.roll(x[b], -r0, axis=0))
        m["condT"], m["tT"] = per_batch[b]
        in_maps.append(m)
    return in_maps


def kernel(**inputs) -> np.ndarray:
    if "nc" not in _CACHED:
        _CACHED["nc"] = build()
    nc = _CACHED["nc"]
    in_maps = _shard_inputs(inputs)
    res = run_bass_kernel_spmd(nc, in_maps, core_ids=list(range(NCORES)))
    outs = [res.results[c]["out"] for c in range(NCORES)]
    full = np.concatenate(outs, axis=0).reshape(B, N, D)
    return full.astype(np.float32)


# revision 51
# speedup vs baseline: 1.0045x; 1.0045x over previous
"""BasicTransformerBlock on 8 TRN2 NeuronCores.

Sharding: sequence-parallel, zero collectives. The [B=2, N=2048, D=512]
residual stream is split into 8 row-blocks of 512 (4 cores per batch
element). Every core recomputes the cheap batch-wide work it needs
(adaln1 + K/V projections over its batch's 2048 rows, cond K/V), and does
attention / FFN only for its own 512 query rows.

Host-side prep (part of kernel()'s sharding layer, shared across cores):
weights are pre-cast to bf16 and pre-rearranged into [partition, ktile,
cols] SBUF layouts and cond is pre-transposed, so every DMA is a fast
contiguous transfer (no software-DGE casting scatter loads) and no on-chip
cond transposes are needed. x stays f32 (layernorm stats need it). Per-core
x is pre-rotated with np.roll so "own" rows are always rows 0:512;
attention is permutation-invariant over keys, so rolled K/V is fine.

Schedule highlights (engine-level):
 - Attention uses transposed scores sT[j, i] so exp() runs on ScalarE
   straight out of PSUM; the 65th v-column of ones makes the softmax
   denominator fall out of the attn@v matmul. Score matmuls are DENSE
   (K=128): both heads of a pair stacked in the stationary operand, query
   operand zero-padded per head. The PE_HAM activity monitor only grants
   the 2.4 GHz clock to full-array matmuls; K=64 scores (even row-tile-
   paired ones) run throttled at 1.2 GHz.
 - The exp stream is the kernel's bottleneck (ScalarE ACTIVATE is
   (N+352)/1.2 ns, ~100us for the 12.6M softmax elements per core), so
   attention is built as one flat software-pipelined granule loop across
   all 4 head pairs: exp(g-1) / scores(g) / attn@v(g-2), with score
   granules alternating 3/2 PSUM slots between two tiles (5 banks) + 2
   accumulator banks + 1 filler bank. Whole-tile rotation expresses the
   2-deep pipeline exactly, so ScalarE runs back-to-back exps while the PE
   fills its slack with the cross-attention K/V projection chains
   (fillers) and softmax tails.
 - adaln: bn_stats chunks -> rstd/nmr -> xn on ScalarE -> PE transpose ->
   fused (1+scale)/shift PSUM drain on DVE, interleaved per 512-row block
   with the dense QKV projection matmuls (PSUM->SBUF drains ride on
   ScalarE Copy where DVE is the busier engine).
 - FFN streams per-ut: zu/zg chains -> Gelu(ScalarE) -> GEGLU stt(DVE) ->
   4 persistent PSUM y-accumulators, output DMA overlapped per row-block.
"""

import contextlib

import ml_dtypes
import numpy as np

import concourse.bass as bass
import concourse.mybir as mybir
import concourse.tile as tile
from concourse import bacc
from concourse.bass_utils import run_bass_kernel_spmd
from concourse.masks import make_identity

dt = mybir.dt
AF = mybir.ActivationFunctionType
OP = mybir.AluOpType

B, N, D = 2, 2048, 512
NCTX = 1024          # cond length
H = 8                # heads
HD = D // H          # 64
EPS = 1e-5
P = 128              # partitions
NCORES = 8
ROWS = 512           # own rows per core
NB = N               # batch rows per core (2048)
SCALE = HD ** -0.5   # 0.125

f32 = dt.float32
bf16 = dt.bfloat16

_CACHED = {}


def _adaln_stats(nc, stat_pool, src_tiles, n_tiles, eps_sb, chunk=4):
    """bn_stats/aggr + rstd/nmr for n_tiles row-tiles. Returns (rstd_all, nmr_all)."""
    mv_all = stat_pool.tile([P, n_tiles, 2], f32)
    rstd_all = stat_pool.tile([P, n_tiles], f32)
    nmr_all = stat_pool.tile([P, n_tiles], f32)
    for c0 in range(0, n_tiles, chunk):
        for it in range(c0, c0 + chunk):
            stats = stat_pool.tile([P, 6], f32, tag="stats")
            nc.vector.bn_stats(stats, src_tiles(it))
            nc.vector.bn_aggr(mv_all[:, it, :], stats)
        cs = slice(c0, c0 + chunk)
        nc.scalar.activation(rstd_all[:, cs], mv_all[:, cs, 1], AF.Sqrt,
                             bias=eps_sb, scale=1.0)
        nc.vector.reciprocal(rstd_all[:, cs], rstd_all[:, cs])
        nc.vector.scalar_tensor_tensor(
            nmr_all[:, cs], mv_all[:, cs, 0], -1.0, rstd_all[:, cs],
            op0=OP.mult, op1=OP.mult,
        )
    return rstd_all, nmr_all


def _adaln_apply_tile(nc, xn_pool, pst_pool, src, it, ab, rstd_all, nmr_all,
                      hT, ident_bf16, on_act=False):
    """One tile: xn = (x-mean)*rstd -> PE transpose -> (1+scale)/shift -> hT."""
    xn = xn_pool.tile([P, 512], bf16, tag="xn")
    if on_act:
        nc.scalar.activation(xn, src, AF.Identity,
                             bias=nmr_all[:, it:it + 1],
                             scale=rstd_all[:, it:it + 1])
    else:
        nc.vector.tensor_scalar(
            xn, src, rstd_all[:, it:it + 1], nmr_all[:, it:it + 1],
            op0=OP.mult, op1=OP.add,
        )
    xnt = pst_pool.tile([P, 4, P], bf16, tag="xnt")
    for b in range(4):
        nc.tensor.transpose(xnt[:, b, :], xn[:, b * P:(b + 1) * P], ident_bf16)
    for b in range(4):
        nc.vector.tensor_scalar(
            hT[:, b, it * P:(it + 1) * P], xnt[:, b, :],
            ab[:, b:b + 1], ab[:, 4 + b:5 + b],
            op0=OP.mult, op1=OP.add,
        )


def _adaln_to_hT(nc, tc, src_tiles, n_tiles, ab, hT, ident_bf16, eps_sb, name):
    with contextlib.ExitStack() as actx:
        stat_pool = actx.enter_context(tc.tile_pool(name=f"{name}_stat", bufs=4))
        xn_pool = actx.enter_context(tc.tile_pool(name=f"{name}_xn", bufs=3))
        pst_pool = actx.enter_context(
            tc.tile_pool(name=f"{name}_pst", bufs=2, space="PSUM"))
        rstd, nmr = _adaln_stats(nc, stat_pool, src_tiles, n_tiles, eps_sb,
                                 chunk=2)
        for it in range(n_tiles):
            _adaln_apply_tile(nc, xn_pool, pst_pool, src_tiles(it), it, ab,
                              rstd, nmr, hT, ident_bf16, on_act=True)


def _attention(nc, tc, act, qpad, kT, v, njt, wo, ob_row, ones_row,
               x_res, x_out, name, fillers=None):
    """Dense-score attention for 8 heads (4 pairs) over own 512 rows.

    qpad: [128, 4, 2, ROWS] bf16 zero-padded per head half. Dense K=128
    score matmuls keep the PE_HAM activity monitor granting 2.4 GHz;
    row-tile-paired K=64 scores measure ~35us more throttle.
    kT:   [128, 4, njt*128] bf16 (partitions = paired head dims).
    v:    [128, njt, 8, 65] bf16 (col 64 of each head = 1.0).
    Writes x_out = attn_out @ wo + ob + x_res  (all [128, 4, 512] f32).

    Granules alternate 3/2 score slots between two PSUM tiles (5 banks),
    leaving one bank for `fillers`: {step: fn(pool)} closures that emit
    independent dense matmul chains into the PE's slack (the exp stream is
    the bottleneck), e.g. the cross-attention K/V projections.
    """
    av_all = act.tile([P, 4, ROWS], bf16, tag="tH")
    S = 2 * njt                       # score slots per pair
    # alternating 3/2-slot granules; G kept EVEN so the two tile tags
    # alternate seamlessly across pair boundaries
    sizes = [3, 2] * 6 + [1, 1] if njt == 16 else [3, 2, 3, 2, 2, 2, 1, 1]
    assert sum(sizes) == S and len(sizes) % 2 == 0
    offs = [0]
    for t in sizes:
        offs.append(offs[-1] + t)
    G = len(sizes)                    # granules per pair
    M = 4 * G                         # global granules across the 4 pairs
    with (
        tc.tile_pool(name=f"{name}_ps_s", bufs=1, space="PSUM") as ps_s,
        tc.tile_pool(name=f"{name}_ps_av", bufs=1, space="PSUM") as ps_av,
        tc.tile_pool(name=f"{name}_ps_f", bufs=1, space="PSUM") as ps_f,
        tc.tile_pool(name=f"{name}_et", bufs=3) as et_pool,
        tc.tile_pool(name=f"{name}_dn", bufs=4) as dn_pool,
        tc.tile_pool(name=f"{name}_rb", bufs=2) as rb_pool,
    ):
        def pair_tail(pht, pavp, rb_ps):
            # softmax denominators: row 64 of each accumulator. Broadcast
            # across partitions with K=1 matmuls into slot 0 of the NEXT
            # score granule's tile (its slot-0 score matmul then overwrites
            # after the reciprocal reads -- read-before-write deps keep it
            # correct), then one fast reciprocal for the pair.
            for hl in range(2):
                dnm = dn_pool.tile([1, ROWS], bf16, tag="dnm")
                nc.vector.tensor_copy(dnm, pavp[HD:HD + 1, hl, :])
                nc.tensor.matmul(
                    rb_ps[hl * HD:(hl + 1) * HD, :],
                    ones_row[0:1, 0:HD], dnm,
                    start=True, stop=True,
                )
            rb_sb = rb_pool.tile([P, ROWS], f32, tag="rb")
            nc.vector.reciprocal_approx_fast(rb_sb, rb_ps)
            for hl in range(2):
                po = hl * HD
                nc.vector.scalar_tensor_tensor(
                    av_all[po:po + HD, pht, :],
                    pavp[0:HD, hl, :], 1.0, rb_sb[po:po + HD, :],
                    op0=OP.mult, op1=OP.mult,
                )

        def lmap(m):
            p, g = divmod(m, G)
            return p, offs[g], offs[g + 1]

        def sg_alloc(m):
            if m % 2 == 0:
                return ps_s.tile([P, 3, ROWS], f32, tag="sgr3", name="sg3")
            return ps_s.tile([P, 2, ROWS], f32, tag="sgr2", name="sg2")

        ets = {}
        sgr = {}
        avps = {}
        pend_sg = None
        for m in range(M + 2):
            if 1 <= m <= M:
                p, lo, hi = lmap(m - 1)
                ng = hi - lo
                et = et_pool.tile([P, 3, ROWS], bf16, tag="et")
                ets[m - 1] = (et, p, lo, hi)
                nc.scalar.activation(
                    et[:, 0:ng, :], sgr[m - 1][:, 0:ng, :], AF.Exp,
                    scale=SCALE,
                )
            if m < M:
                p, lo, hi = lmap(m)
                if pend_sg is not None:
                    sg = pend_sg
                    pend_sg = None
                else:
                    sg = sg_alloc(m)
                sgr[m] = sg
                for s in range(lo, hi):
                    jt, hl = s // 2, s % 2
                    nc.tensor.matmul(
                        sg[:, s - lo, :],
                        kT[:, p, jt * P:(jt + 1) * P],
                        qpad[:, p, hl, :],
                        start=True, stop=True,
                    )
            if m >= 2:
                et, p, lo, hi = ets.pop(m - 2)
                sgr.pop(m - 2, None)
                if p not in avps:
                    avps[p] = ps_av.tile([P, 2, ROWS], f32, tag="av",
                                         name=f"avp{p}")
                for i, s in enumerate(range(lo, hi)):
                    jt, hl = s // 2, s % 2
                    nc.tensor.matmul(
                        avps[p][0:HD + 1, hl, :], v[:, jt, 2 * p + hl, :],
                        et[:, i, :],
                        start=(jt == 0), stop=(jt == njt - 1),
                    )
                if hi == S:  # pair p's accumulators complete: emit its tail
                    pend_sg = sg_alloc(m + 1)   # granule consumed next step
                    pair_tail(p, avps[p], pend_sg[:, 0, :])
            if fillers and m in fillers:
                fillers[m](ps_f)
    # out-projection + bias + residual
    with tc.tile_pool(name=f"{name}_ps_o", bufs=2, space="PSUM") as ps_o:
        for it in range(4):
            ps = ps_o.tile([P, D], f32, tag="o")
            for dt_ in range(4):
                nc.tensor.matmul(
                    ps, av_all[:, dt_, it * P:(it + 1) * P], wo[:, dt_, :],
                    start=(dt_ == 0), stop=False,
                )
            nc.tensor.matmul(
                ps, ones_row[0:1, 0:P], ob_row, start=False, stop=True,
            )
            nc.vector.tensor_tensor(x_out[:, it, :], ps, x_res[:, it, :], op=OP.add)


def build():
    nc = bacc.Bacc(None, target_bir_lowering=False)

    # -------- I/O (weights arrive pre-cast/pre-arranged from the host) ----
    xb = nc.dram_tensor("xb", [NB, D], f32, kind="ExternalInput")
    condT_d = nc.dram_tensor("condT", [P, 4, NCTX], bf16, kind="ExternalInput")
    tT_d = nc.dram_tensor("tT", [P, 4], bf16, kind="ExternalInput")
    nw_d = {}
    nb_d = {}
    for l in (1, 2, 4):
        nw_d[l] = nc.dram_tensor(f"nw{l}", [P, 4, 2 * D], bf16,
                                 kind="ExternalInput")
        nb_d[l] = nc.dram_tensor(f"nb{l}", [2 * D], f32, kind="ExternalInput")
    as_d = {}
    aob_d = {}
    for a in (1, 2):
        as_d[a] = nc.dram_tensor(f"a{a}s", [P, 4, 4, D], bf16,
                                 kind="ExternalInput")
        aob_d[a] = nc.dram_tensor(f"a{a}ob", [D], bf16, kind="ExternalInput")
    ffw1_d = nc.dram_tensor("ffw1", [P, 4, 8 * D], bf16, kind="ExternalInput")
    ffw2_d = nc.dram_tensor("ffw2", [P, 16, D], bf16, kind="ExternalInput")
    ffb1_d = nc.dram_tensor("ffb1", [P, 32], f32, kind="ExternalInput")
    ffb2_d = nc.dram_tensor("ffb2", [D], bf16, kind="ExternalInput")
    out = nc.dram_tensor("out", [ROWS, D], f32, kind="ExternalOutput")

    with tile.TileContext(nc) as tc, contextlib.ExitStack() as ctx:
        const = ctx.enter_context(tc.tile_pool(name="const", bufs=1))
        wpool = ctx.enter_context(tc.tile_pool(name="wpool", bufs=1))
        act = ctx.enter_context(tc.tile_pool(name="act", bufs=1))
        xr_pool = ctx.enter_context(tc.tile_pool(name="xrp", bufs=6))
        n1_stat = ctx.enter_context(tc.tile_pool(name="n1_stat", bufs=4))

        ident_bf16 = const.tile([P, P], bf16)
        make_identity(nc, ident_bf16)
        ident_f32 = const.tile([P, P], f32)
        make_identity(nc, ident_f32)
        ones_row = const.tile([1, P], bf16)
        nc.vector.memset(ones_row, 1.0)
        eps_sb = const.tile([P, 1], f32)
        nc.vector.memset(eps_sb, EPS)

        # ---------------- DMA issue order --------------------------------
        # qACT: weights in first-use order.  qSP: x, small rows, ff tail.
        tT = const.tile([P, 4], bf16)
        nc.scalar.dma_start(tT, tT_d[:])
        ab = {}
        with (
            tc.tile_pool(name="nwp", bufs=1) as nwp,
            tc.tile_pool(name="embp", bufs=1) as embp,
        ):
            nw_sb = {}
            for l in (1, 2):
                nw_sb[l] = nwp.tile([P, 4, 2 * D], bf16, tag=f"nw{l}",
                                    name=f"nw_sb{l}")
                nc.scalar.dma_start(nw_sb[l], nw_d[l][:])

            a_sb = {}
            stacks = {}
            for a, wtag in ((1, "wbig1"), (2, "wbig2")):
                stack = wpool.tile([P, 4, 4, D], bf16, tag=wtag,
                                   name=f"a{a}stk")
                stacks[a] = stack
                for wi, w in enumerate("qkvo"):
                    a_sb[a, w] = stack[:, :, wi, :]
            pass
            for a in (1, 2):
                ob = wpool.tile([1, D], bf16, tag=f"a{a}ob", name=f"a{a}ob_sb")
                a_sb[a, "ob"] = ob

            h1T = act.tile([P, 4, NB], bf16, tag="tA")
            own_x = act.tile([P, 4, D], f32, tag="tE")
            x_tiles = {}
            for it in range(16):
                if it < 4:
                    dst = own_x[:, it, :]
                else:
                    dst = xr_pool.tile([P, D], f32, tag="xr", name=f"xr{it}")
                nc.gpsimd.dma_start(dst, xb[:][it * P:(it + 1) * P, :])
                x_tiles[it] = dst

            # condT + k2T share the tX region
            ctk2 = act.tile([P, 2, 4, NCTX], bf16, tag="tX")
            condT = ctk2[:, 0, :, :]
            k2T = ctk2[:, 1, :, :]
            # big weights + condT on the otherwise-idle SWDGE queue: HWDGE
            # trigger instructions cost ~2.5us EACH on their engine's queue
            # and were starving the ACT stats chain.
            nc.gpsimd.dma_start(stacks[1], as_d[1][:])
            nc.gpsimd.dma_start(a_sb[1, "ob"],
                                aob_d[1][:].rearrange("(a n) -> a n", a=1))
            nc.gpsimd.dma_start(stacks[2], as_d[2][:])
            nc.gpsimd.dma_start(a_sb[2, "ob"],
                                aob_d[2][:].rearrange("(a n) -> a n", a=1))
            nc.gpsimd.dma_start(condT, condT_d[:])
            nw_sb[4] = nwp.tile([P, 4, 2 * D], bf16, tag="nw1", name="nw_sb4")
            nc.gpsimd.dma_start(nw_sb[4], nw_d[4][:])

            nb_row = {}
            for l in (1, 2, 4):
                nb_row[l] = embp.tile([1, 2 * D], f32, tag="nbrow",
                                      name=f"nb_row{l}")
                nc.sync.dma_start(nb_row[l],
                                  nb_d[l][:].rearrange("(a n) -> a n", a=1))
            b1_sb = const.tile([P, 32], f32)
            nc.sync.dma_start(b1_sb, ffb1_d[:])
            b2_row = const.tile([1, D], bf16)
            nc.sync.dma_start(b2_row, ffb2_d[:].rearrange("(a n) -> a n", a=1))
            # ff weights on qSP after x: landed long before the FFN needs
            # them, but the buffers alias a1s/a2s so they wait on attention.
            w1_sb = wpool.tile([P, 4, 8 * D], bf16, tag="wbig1")
            nc.gpsimd.dma_start(w1_sb, ffw1_d[:])
            w2_sb = wpool.tile([P, 16, D], bf16, tag="wbig2")
            nc.gpsimd.dma_start(w2_sb, ffw2_d[:])

            # PE warmup: dependency-free matmuls fill the startup DMA window
            with tc.tile_pool(name="warm", bufs=1, space="PSUM") as warm_pool:
                wps = warm_pool.tile([P, P], f32)
                for _ in range(30):
                    nc.tensor.matmul(wps, ident_bf16, ident_bf16,
                                     start=True, stop=True)

            # ------------- norm scale/shift params -----------------------
            # only ab[1] gates the adaln1 loop; l=2,4 run after it so their
            # nw DMA arrival never stalls the PE stream.
            def emb_ab(l, ps_emb):
                emb_ps = ps_emb.tile([1, 2 * D], f32, tag="embps",
                                     name=f"emb_ps{l}")
                for half in range(2):
                    for kt in range(4):
                        nc.tensor.matmul(
                            emb_ps[:, half * D:(half + 1) * D],
                            tT[:, kt:kt + 1],
                            nw_sb[l][:, kt, half * D:(half + 1) * D],
                            start=(kt == 0), stop=(kt == 3),
                        )
                emb_row = embp.tile([1, 2 * D], f32, tag="embrow",
                                    name=f"emb_row{l}")
                nc.vector.tensor_tensor(emb_row, emb_ps, nb_row[l], op=OP.add)
                ab_l = const.tile([P, 8], f32, tag=f"ab{l}", name=f"ab_{l}")
                for col in range(8):
                    tp = ps_emb.tile([P, 1], f32, tag="embT")
                    nc.tensor.transpose(
                        tp, emb_row[0:1, col * P:(col + 1) * P],
                        ident_f32[0:1, 0:1]
                    )
                    nc.vector.tensor_scalar(
                        ab_l[:, col:col + 1], tp,
                        1.0 if col < 4 else 0.0, None, op0=OP.add,
                    )
                ab[l] = ab_l

            with tc.tile_pool(name="ps_emb1", bufs=2, space="PSUM") as pe1:
                emb_ab(1, pe1)

            # --------- adaln1 apply + projections, interleaved -----------
            # Per 512-row block: stats chunk -> xn (ACT) -> PE transpose ->
            # affine (DVE) -> dense k1/v1/q matmuls, with the PSUM->SBUF
            # drains on the otherwise-idle ACT engine.
            k1T = act.tile([P, 4, NB], bf16, tag="tB")
            v1 = act.tile([P, 16, H, HD + 1], bf16, tag="tC")
            q1pad = act.tile([P, 4, 2, ROWS], bf16, tag="tD")
            nc.gpsimd.memset(v1[:, :, :, HD:HD + 1], 1.0)
            nc.gpsimd.memset(q1pad, 0.0)
            mv1 = n1_stat.tile([P, 16, 2], f32)
            rstd1 = n1_stat.tile([P, 16], f32)
            nmr1 = n1_stat.tile([P, 16], f32)
            v2 = act.tile([P, 8, H, HD + 1], bf16, tag="tI")
            nc.gpsimd.memset(v2[:, :, :, HD:HD + 1], 1.0)
            with (
                tc.tile_pool(name="n1_xn", bufs=3) as xn_pool,
                tc.tile_pool(name="n1_pst", bufs=2, space="PSUM") as pst_pool,
                tc.tile_pool(name="ps_proj1", bufs=4, space="PSUM") as ps_proj,
            ):
                for jc in range(4):
                    cs = slice(4 * jc, 4 * jc + 4)
                    for it in range(4 * jc, 4 * jc + 4):
                        stats = n1_stat.tile([P, 6], f32, tag="stats")
                        nc.vector.bn_stats(stats, x_tiles[it])
                        nc.vector.bn_aggr(mv1[:, it, :], stats)
                    nc.scalar.activation(rstd1[:, cs], mv1[:, cs, 1], AF.Sqrt,
                                         bias=eps_sb, scale=1.0)
                    nc.vector.reciprocal(rstd1[:, cs], rstd1[:, cs])
                    nc.vector.scalar_tensor_tensor(
                        nmr1[:, cs], mv1[:, cs, 0], -1.0, rstd1[:, cs],
                        op0=OP.mult, op1=OP.mult,
                    )
                    for it in range(4 * jc, 4 * jc + 4):
                        _adaln_apply_tile(nc, xn_pool, pst_pool, x_tiles[it],
                                          it, ab[1], rstd1, nmr1, h1T,
                                          ident_bf16, on_act=True)
                    for dt_ in range(4):
                        ps = ps_proj.tile([P, 512], f32, tag="proj")
                        for kt in range(4):
                            nc.tensor.matmul(
                                ps,
                                a_sb[1, "k"][:, kt, dt_ * P:(dt_ + 1) * P],
                                h1T[:, kt, jc * 512:(jc + 1) * 512],
                                start=(kt == 0), stop=(kt == 3),
                            )
                        nc.scalar.activation(
                            k1T[:, dt_, jc * 512:(jc + 1) * 512], ps, AF.Copy
                        )
                    for jt in range(4 * jc, 4 * jc + 4):
                        ps = ps_proj.tile([P, 512], f32, tag="proj")
                        for kt in range(4):
                            nc.tensor.matmul(
                                ps,
                                h1T[:, kt, jt * P:(jt + 1) * P],
                                a_sb[1, "v"][:, kt, :],
                                start=(kt == 0), stop=(kt == 3),
                            )
                        nc.scalar.activation(
                            v1[:, jt, :, 0:HD],
                            ps.rearrange("p (h d) -> p h d", h=H), AF.Copy
                        )
                    if jc == 0:
                        for dt_ in range(4):
                            ps = ps_proj.tile([P, 512], f32, tag="proj")
                            for kt in range(4):
                                nc.tensor.matmul(
                                    ps,
                                    a_sb[1, "q"][:, kt, dt_ * P:(dt_ + 1) * P],
                                    h1T[:, kt, 0:ROWS],
                                    start=(kt == 0), stop=(kt == 3),
                                )
                            nc.scalar.activation(q1pad[0:HD, dt_, 0, :],
                                                 ps[0:HD, :], AF.Copy)
                            nc.scalar.activation(q1pad[HD:P, dt_, 1, :],
                                                 ps[HD:P, :], AF.Copy)

            with tc.tile_pool(name="ps_emb2", bufs=2, space="PSUM") as pe2:
                emb_ab(2, pe2)
                emb_ab(4, pe2)

        # ---------------- attention 1 ------------------------------------
        # cross-attn K/V projections ride along as fillers in attn1's PE
        # slack (the exp stream is the bottleneck there); each chain uses
        # the one spare PSUM bank.
        def mk_k2(dt_, cjc):
            def f(pool):
                ps = pool.tile([P, 512], f32, tag="fps", name="fps")
                for kt in range(4):
                    nc.tensor.matmul(
                        ps,
                        a_sb[2, "k"][:, kt, dt_ * P:(dt_ + 1) * P],
                        condT[:, kt, cjc * 512:(cjc + 1) * 512],
                        start=(kt == 0), stop=(kt == 3),
                    )
                nc.vector.tensor_copy(
                    k2T[:, dt_, cjc * 512:(cjc + 1) * 512], ps
                )
            return f

        def mk_v2(jt):
            def f(pool):
                ps = pool.tile([P, 512], f32, tag="fps", name="fps")
                for kt in range(4):
                    nc.tensor.matmul(
                        ps,
                        condT[:, kt, jt * P:(jt + 1) * P],
                        a_sb[2, "v"][:, kt, :],
                        start=(kt == 0), stop=(kt == 3),
                    )
                nc.vector.tensor_copy(
                    v2[:, jt, :, 0:HD], ps.rearrange("p (h d) -> p h d", h=H)
                )
            return f

        fill1 = {}
        fns = ([mk_k2(dt_, cjc) for dt_ in range(4) for cjc in range(2)]
               + [mk_v2(jt) for jt in range(8)])
        for i, fn in enumerate(fns):
            fill1[20 + 2 * i] = fn

        x2 = act.tile([P, 4, D], f32, tag="tF")
        _attention(nc, tc, act, q1pad, k1T, v1, 16, a_sb[1, "o"],
                   a_sb[1, "ob"], ones_row, own_x, x2, "att1",
                   fillers=fill1)

        # ---------------- adaln2 + cross-attn ----------------------------
        h2T = act.tile([P, 4, ROWS], bf16, tag="tH")
        _adaln_to_hT(nc, tc, lambda it: x2[:, it, :], 4, ab[2], h2T,
                     ident_bf16, eps_sb, "n2")

        q2pad = act.tile([P, 4, 2, ROWS], bf16, tag="tD")
        nc.gpsimd.memset(q2pad, 0.0)
        with tc.tile_pool(name="ps_proj2b", bufs=2, space="PSUM") as ps_proj:
            for dt_ in range(4):
                ps = ps_proj.tile([P, 512], f32, tag="proj")
                for kt in range(4):
                    nc.tensor.matmul(
                        ps,
                        a_sb[2, "q"][:, kt, dt_ * P:(dt_ + 1) * P],
                        h2T[:, kt, :],
                        start=(kt == 0), stop=(kt == 3),
                    )
                nc.scalar.activation(q2pad[0:HD, dt_, 0, :], ps[0:HD, :],
                                     AF.Copy)
                nc.scalar.activation(q2pad[HD:P, dt_, 1, :], ps[HD:P, :],
                                     AF.Copy)

        x3 = act.tile([P, 4, D], f32, tag="tG")
        _attention(nc, tc, act, q2pad, k2T, v2, 8, a_sb[2, "o"],
                   a_sb[2, "ob"], ones_row, x2, x3, "att2")

        # ---------------- adaln3 + GEGLU FFN -----------------------------
        h3T = act.tile([P, 4, ROWS], bf16, tag="tJ")
        _adaln_to_hT(nc, tc, lambda it: x3[:, it, :], 4, ab[4], h3T,
                     ident_bf16, eps_sb, "n4")

        # per-ut pipeline: zu/zg -> gelu/stt -> 4 y-accumulator matmuls.
        # y accumulates in 4 persistent PSUM banks across all 16 ut chunks.
        ugT = act.tile([P, 16, ROWS], bf16, tag="tA")
        out_sb = act.tile([P, 4, D], f32, tag="tC")
        with (
            tc.tile_pool(name="ps_z", bufs=4, space="PSUM") as ps_z,
            tc.tile_pool(name="ps_y", bufs=1, space="PSUM") as ps_y,
            tc.tile_pool(name="gact", bufs=3) as gact_pool,
        ):
            y_ps = ps_y.tile([P, 4, D], f32)
            for ut in range(16):
                zu = ps_z.tile([P, ROWS], f32, tag="z")
                zg = ps_z.tile([P, ROWS], f32, tag="z")
                for kt in range(4):
                    nc.tensor.matmul(
                        zu, w1_sb[:, kt, ut * P:(ut + 1) * P],
                        h3T[:, kt, :], start=(kt == 0), stop=(kt == 3),
                    )
                for kt in range(4):
                    nc.tensor.matmul(
                        zg, w1_sb[:, kt, (16 + ut) * P:(17 + ut) * P],
                        h3T[:, kt, :], start=(kt == 0), stop=(kt == 3),
                    )
                gact = gact_pool.tile([P, ROWS], bf16, tag="gact")
                nc.scalar.activation(
                    gact, zg, AF.Gelu, bias=b1_sb[:, 16 + ut:17 + ut], scale=1.0
                )
                nc.vector.scalar_tensor_tensor(
                    ugT[:, ut, :], zu, b1_sb[:, ut:ut + 1], gact,
                    op0=OP.add, op1=OP.mult,
                )
                for it in range(4):
                    nc.tensor.matmul(
                        y_ps[:, it, :], ugT[:, ut, it * P:(it + 1) * P],
                        w2_sb[:, ut, :],
                        start=(ut == 0), stop=False,
                    )
            for it in range(4):
                nc.tensor.matmul(
                    y_ps[:, it, :], ones_row[0:1, 0:P], b2_row,
                    start=False, stop=True,
                )
                nc.vector.tensor_tensor(
                    out_sb[:, it, :], y_ps[:, it, :], x3[:, it, :], op=OP.add
                )
                nc.sync.dma_start(out[:][it * P:(it + 1) * P, :],
                                  out_sb[:, it, :])

    nc.compile()
    return nc


def _prep_shared(inputs):
    """Pre-cast weights to bf16 and pre-arrange into SBUF layouts (host-side
    layout prep, shared by all cores)."""
    bf = ml_dtypes.bfloat16

    def pkn(w, ktiles):
        # [ktiles*128, n] f32 -> [128, ktiles, n] bf16
        n = w.shape[1]
        return np.ascontiguousarray(
            w.reshape(ktiles, P, n).transpose(1, 0, 2).astype(bf))

    shared = {}
    for l in (1, 2, 4):
        shared[f"nw{l}"] = pkn(np.asarray(inputs[f"n{l}_w"], np.float32), 4)
        shared[f"nb{l}"] = np.ascontiguousarray(inputs[f"n{l}_b"], np.float32)
    for a in (1, 2):
        ws = [pkn(np.asarray(inputs[f"a{a}_{w}"], np.float32), 4)
              for w in "qkvo"]
        shared[f"a{a}s"] = np.ascontiguousarray(np.stack(ws, axis=2))
        shared[f"a{a}ob"] = np.asarray(inputs[f"a{a}_ob"], np.float32).astype(bf)
    shared["ffw1"] = pkn(np.asarray(inputs["ff_w1"], np.float32), 4)
    shared["ffw2"] = pkn(np.asarray(inputs["ff_w2"], np.float32), 16)
    shared["ffb1"] = np.ascontiguousarray(
        np.asarray(inputs["ff_b1"], np.float32).reshape(32, P).T)
    shared["ffb2"] = np.asarray(inputs["ff_b2"], np.float32).astype(bf)
    return shared


def _shard_inputs(inputs):
    """Build the 8 per-core input maps."""
    bf = ml_dtypes.bfloat16
    x = np.ascontiguousarray(inputs["x"], dtype=np.float32)
    t = np.ascontiguousarray(inputs["t"], dtype=np.float32)
    cond = np.ascontiguousarray(inputs["cond"], dtype=np.float32)
    shared = _prep_shared(inputs)
    per_batch = {}
    for b in range(B):
        condT = cond[b].T.reshape(4, P, NCTX).transpose(1, 0, 2)
        tT = t[b, 0].reshape(4, P).T
        per_batch[b] = (
            np.ascontiguousarray(condT.astype(bf)),
            np.ascontiguousarray(tT.astype(bf)),
        )
    in_maps = []
    for c in range(NCORES):
        b = c // 4
        r0 = (c % 4) * ROWS
        m = dict(shared)
        m["xb"] = np.ascontiguousarray(np.roll(x[b], -r0, axis=0))
        m["condT"], m["tT"] = per_batch[b]
        in_maps.append(m)
    return in_maps


def kernel(**inputs) -> np.ndarray:
    if "nc" not in _CACHED:
        _CACHED["nc"] = build()
    nc = _CACHED["nc"]
    in_maps = _shard_inputs(inputs)
    res = run_bass_kernel_spmd(nc, in_maps, core_ids=list(range(NCORES)))
    outs = [res.results[c]["out"] for c in range(NCORES)]
    full = np.concatenate(outs, axis=0).reshape(B, N, D)
    return full.astype(np.float32)


# revision 52
# speedup vs baseline: 1.0079x; 1.0034x over previous
"""BasicTransformerBlock on 8 TRN2 NeuronCores.

Sharding: sequence-parallel, zero collectives. The [B=2, N=2048, D=512]
residual stream is split into 8 row-blocks of 512 (4 cores per batch
element). Every core recomputes the cheap batch-wide work it needs
(adaln1 + K/V projections over its batch's 2048 rows, cond K/V), and does
attention / FFN only for its own 512 query rows.

Host-side prep (part of kernel()'s sharding layer, shared across cores):
weights are pre-cast to bf16 and pre-rearranged into [partition, ktile,
cols] SBUF layouts and cond is pre-transposed, so every DMA is a fast
contiguous transfer (no software-DGE casting scatter loads) and no on-chip
cond transposes are needed. x stays f32 (layernorm stats need it). Per-core
x is pre-rotated with np.roll so "own" rows are always rows 0:512;
attention is permutation-invariant over keys, so rolled K/V is fine.

Schedule highlights (engine-level):
 - Attention uses transposed scores sT[j, i] so exp() runs on ScalarE
   straight out of PSUM; the 65th v-column of ones makes the softmax
   denominator fall out of the attn@v matmul. Score matmuls are DENSE
   (K=128): both heads of a pair stacked in the stationary operand, query
   operand zero-padded per head. The PE_HAM activity monitor only grants
   the 2.4 GHz clock to full-array matmuls; K=64 scores (even row-tile-
   paired ones) run throttled at 1.2 GHz.
 - The exp stream is the kernel's bottleneck (ScalarE ACTIVATE is
   (N+352)/1.2 ns, ~100us for the 12.6M softmax elements per core), so
   attention is built as one flat software-pipelined granule loop across
   all 4 head pairs: exp(g-1) / scores(g) / attn@v(g-2), with score
   granules alternating 3/2 PSUM slots between two tiles (5 banks) + 2
   accumulator banks + 1 filler bank. Whole-tile rotation expresses the
   2-deep pipeline exactly, so ScalarE runs back-to-back exps while the PE
   fills its slack with the cross-attention K/V projection chains
   (fillers) and softmax tails.
 - adaln: bn_stats chunks -> rstd/nmr -> xn on ScalarE -> PE transpose ->
   fused (1+scale)/shift PSUM drain on DVE, interleaved per 512-row block
   with the dense QKV projection matmuls (PSUM->SBUF drains ride on
   ScalarE Copy where DVE is the busier engine).
 - FFN streams per-ut: zu/zg chains -> Gelu(ScalarE) -> GEGLU stt(DVE) ->
   4 persistent PSUM y-accumulators, output DMA overlapped per row-block.
"""

import contextlib

import ml_dtypes
import numpy as np

import concourse.bass as bass
import concourse.mybir as mybir
import concourse.tile as tile
from concourse import bacc
from concourse.bass_utils import run_bass_kernel_spmd
from concourse.masks import make_identity

dt = mybir.dt
AF = mybir.ActivationFunctionType
OP = mybir.AluOpType

B, N, D = 2, 2048, 512
NCTX = 1024          # cond length
H = 8                # heads
HD = D // H          # 64
EPS = 1e-5
P = 128              # partitions
NCORES = 8
ROWS = 512           # own rows per core
NB = N               # batch rows per core (2048)
SCALE = HD ** -0.5   # 0.125

f32 = dt.float32
bf16 = dt.bfloat16

_CACHED = {}


def _adaln_stats(nc, stat_pool, src_tiles, n_tiles, eps_sb, chunk=4):
    """bn_stats/aggr + rstd/nmr for n_tiles row-tiles. Returns (rstd_all, nmr_all)."""
    mv_all = stat_pool.tile([P, n_tiles, 2], f32)
    rstd_all = stat_pool.tile([P, n_tiles], f32)
    nmr_all = stat_pool.tile([P, n_tiles], f32)
    for c0 in range(0, n_tiles, chunk):
        for it in range(c0, c0 + chunk):
            stats = stat_pool.tile([P, 6], f32, tag="stats")
            nc.vector.bn_stats(stats, src_tiles(it))
            nc.vector.bn_aggr(mv_all[:, it, :], stats)
        cs = slice(c0, c0 + chunk)
        nc.scalar.activation(rstd_all[:, cs], mv_all[:, cs, 1], AF.Sqrt,
                             bias=eps_sb, scale=1.0)
        nc.vector.reciprocal(rstd_all[:, cs], rstd_all[:, cs])
        nc.vector.scalar_tensor_tensor(
            nmr_all[:, cs], mv_all[:, cs, 0], -1.0, rstd_all[:, cs],
            op0=OP.mult, op1=OP.mult,
        )
    return rstd_all, nmr_all


def _adaln_apply_tile(nc, xn_pool, pst_pool, src, it, ab, rstd_all, nmr_all,
                      hT, ident_bf16, on_act=False):
    """One tile: xn = (x-mean)*rstd -> PE transpose -> (1+scale)/shift -> hT."""
    xn = xn_pool.tile([P, 512], bf16, tag="xn")
    if on_act:
        nc.scalar.activation(xn, src, AF.Identity,
                             bias=nmr_all[:, it:it + 1],
                             scale=rstd_all[:, it:it + 1])
    else:
        nc.vector.tensor_scalar(
            xn, src, rstd_all[:, it:it + 1], nmr_all[:, it:it + 1],
            op0=OP.mult, op1=OP.add,
        )
    xnt = pst_pool.tile([P, 4, P], bf16, tag="xnt")
    for b in range(4):
        nc.tensor.transpose(xnt[:, b, :], xn[:, b * P:(b + 1) * P], ident_bf16)
    for b in range(4):
        nc.vector.tensor_scalar(
            hT[:, b, it * P:(it + 1) * P], xnt[:, b, :],
            ab[:, b:b + 1], ab[:, 4 + b:5 + b],
            op0=OP.mult, op1=OP.add,
        )


def _adaln_to_hT(nc, tc, src_tiles, n_tiles, ab, hT, ident_bf16, eps_sb, name):
    with contextlib.ExitStack() as actx:
        stat_pool = actx.enter_context(tc.tile_pool(name=f"{name}_stat", bufs=4))
        xn_pool = actx.enter_context(tc.tile_pool(name=f"{name}_xn", bufs=3))
        pst_pool = actx.enter_context(
            tc.tile_pool(name=f"{name}_pst", bufs=2, space="PSUM"))
        rstd, nmr = _adaln_stats(nc, stat_pool, src_tiles, n_tiles, eps_sb,
                                 chunk=2)
        for it in range(n_tiles):
            _adaln_apply_tile(nc, xn_pool, pst_pool, src_tiles(it), it, ab,
                              rstd, nmr, hT, ident_bf16, on_act=True)


def _attention(nc, tc, act, qpad, kT, v, njt, wo, ob_row, ones_row,
               x_res, x_out, name, fillers=None):
    """Dense-score attention for 8 heads (4 pairs) over own 512 rows.

    qpad: [128, 4, 2, ROWS] bf16 zero-padded per head half. Dense K=128
    score matmuls keep the PE_HAM activity monitor granting 2.4 GHz;
    row-tile-paired K=64 scores measure ~35us more throttle.
    kT:   [128, 4, njt*128] bf16 (partitions = paired head dims).
    v:    [128, njt, 8, 65] bf16 (col 64 of each head = 1.0).
    Writes x_out = attn_out @ wo + ob + x_res  (all [128, 4, 512] f32).

    Granules alternate 3/2 score slots between two PSUM tiles (5 banks),
    leaving one bank for `fillers`: {step: fn(pool)} closures that emit
    independent dense matmul chains into the PE's slack (the exp stream is
    the bottleneck), e.g. the cross-attention K/V projections.
    """
    av_all = act.tile([P, 4, ROWS], bf16, tag="tH")
    S = 2 * njt                       # score slots per pair
    # alternating 3/2-slot granules; G kept EVEN so the two tile tags
    # alternate seamlessly across pair boundaries
    sizes = [3, 2] * 6 + [1, 1] if njt == 16 else [3, 2, 3, 2, 2, 2, 1, 1]
    assert sum(sizes) == S and len(sizes) % 2 == 0
    offs = [0]
    for t in sizes:
        offs.append(offs[-1] + t)
    G = len(sizes)                    # granules per pair
    M = 4 * G                         # global granules across the 4 pairs
    with (
        tc.tile_pool(name=f"{name}_ps_s", bufs=1, space="PSUM") as ps_s,
        tc.tile_pool(name=f"{name}_ps_av", bufs=1, space="PSUM") as ps_av,
        tc.tile_pool(name=f"{name}_ps_f", bufs=1, space="PSUM") as ps_f,
        tc.tile_pool(name=f"{name}_et", bufs=3) as et_pool,
        tc.tile_pool(name=f"{name}_dn", bufs=4) as dn_pool,
        tc.tile_pool(name=f"{name}_rb", bufs=2) as rb_pool,
    ):
        def pair_tail(pht, pavp, rb_ps):
            # softmax denominators: row 64 of each accumulator. Broadcast
            # across partitions with K=1 matmuls into slot 0 of the NEXT
            # score granule's tile (its slot-0 score matmul then overwrites
            # after the reciprocal reads -- read-before-write deps keep it
            # correct), then one fast reciprocal for the pair.
            for hl in range(2):
                dnm = dn_pool.tile([1, ROWS], bf16, tag="dnm")
                nc.vector.tensor_copy(dnm, pavp[HD:HD + 1, hl, :])
                nc.tensor.matmul(
                    rb_ps[hl * HD:(hl + 1) * HD, :],
                    ones_row[0:1, 0:HD], dnm,
                    start=True, stop=True,
                )
            rb_sb = rb_pool.tile([P, ROWS], f32, tag="rb")
            nc.vector.reciprocal_approx_fast(rb_sb, rb_ps)
            for hl in range(2):
                po = hl * HD
                nc.vector.scalar_tensor_tensor(
                    av_all[po:po + HD, pht, :],
                    pavp[0:HD, hl, :], 1.0, rb_sb[po:po + HD, :],
                    op0=OP.mult, op1=OP.mult,
                )

        def lmap(m):
            p, g = divmod(m, G)
            return p, offs[g], offs[g + 1]

        def sg_alloc(m):
            if m % 2 == 0:
                return ps_s.tile([P, 3, ROWS], f32, tag="sgr3", name="sg3")
            return ps_s.tile([P, 2, ROWS], f32, tag="sgr2", name="sg2")

        ets = {}
        sgr = {}
        avps = {}
        pend_sg = None
        for m in range(M + 2):
            if 1 <= m <= M:
                p, lo, hi = lmap(m - 1)
                ng = hi - lo
                et = et_pool.tile([P, 3, ROWS], bf16, tag="et")
                ets[m - 1] = (et, p, lo, hi)
                nc.scalar.activation(
                    et[:, 0:ng, :], sgr[m - 1][:, 0:ng, :], AF.Exp,
                    scale=SCALE,
                )
            if m < M:
                p, lo, hi = lmap(m)
                if pend_sg is not None:
                    sg = pend_sg
                    pend_sg = None
                else:
                    sg = sg_alloc(m)
                sgr[m] = sg
                for s in range(lo, hi):
                    jt, hl = s // 2, s % 2
                    nc.tensor.matmul(
                        sg[:, s - lo, :],
                        kT[:, p, jt * P:(jt + 1) * P],
                        qpad[:, p, hl, :],
                        start=True, stop=True,
                    )
            if m >= 2:
                et, p, lo, hi = ets.pop(m - 2)
                sgr.pop(m - 2, None)
                if p not in avps:
                    avps[p] = ps_av.tile([P, 2, ROWS], f32, tag="av",
                                         name=f"avp{p}")
                for i, s in enumerate(range(lo, hi)):
                    jt, hl = s // 2, s % 2
                    nc.tensor.matmul(
                        avps[p][0:HD + 1, hl, :], v[:, jt, 2 * p + hl, :],
                        et[:, i, :],
                        start=(jt == 0), stop=(jt == njt - 1),
                    )
                if hi == S:  # pair p's accumulators complete: emit its tail
                    pend_sg = sg_alloc(m + 1)   # granule consumed next step
                    pair_tail(p, avps[p], pend_sg[:, 0, :])
            if fillers and m in fillers:
                fillers[m](ps_f)
    # out-projection + bias + residual
    with tc.tile_pool(name=f"{name}_ps_o", bufs=2, space="PSUM") as ps_o:
        for it in range(4):
            ps = ps_o.tile([P, D], f32, tag="o")
            for dt_ in range(4):
                nc.tensor.matmul(
                    ps, av_all[:, dt_, it * P:(it + 1) * P], wo[:, dt_, :],
                    start=(dt_ == 0), stop=False,
                )
            nc.tensor.matmul(
                ps, ones_row[0:1, 0:P], ob_row, start=False, stop=True,
            )
            nc.vector.tensor_tensor(x_out[:, it, :], ps, x_res[:, it, :], op=OP.add)


def build():
    nc = bacc.Bacc(None, target_bir_lowering=False)

    # -------- I/O (weights arrive pre-cast/pre-arranged from the host) ----
    xb = nc.dram_tensor("xb", [NB, D], f32, kind="ExternalInput")
    condT_d = nc.dram_tensor("condT", [P, 4, NCTX], bf16, kind="ExternalInput")
    tT_d = nc.dram_tensor("tT", [P, 4], bf16, kind="ExternalInput")
    nw_d = {}
    nb_d = {}
    for l in (1, 2, 4):
        nw_d[l] = nc.dram_tensor(f"nw{l}", [P, 4, 2 * D], bf16,
                                 kind="ExternalInput")
        nb_d[l] = nc.dram_tensor(f"nb{l}", [2 * D], f32, kind="ExternalInput")
    as_d = {}
    aob_d = {}
    for a in (1, 2):
        as_d[a] = nc.dram_tensor(f"a{a}s", [P, 4, 4, D], bf16,
                                 kind="ExternalInput")
        aob_d[a] = nc.dram_tensor(f"a{a}ob", [D], bf16, kind="ExternalInput")
    ffw1_d = nc.dram_tensor("ffw1", [P, 4, 8 * D], bf16, kind="ExternalInput")
    ffw2_d = nc.dram_tensor("ffw2", [P, 16, D], bf16, kind="ExternalInput")
    ffb1_d = nc.dram_tensor("ffb1", [P, 32], f32, kind="ExternalInput")
    ffb2_d = nc.dram_tensor("ffb2", [D], bf16, kind="ExternalInput")
    out = nc.dram_tensor("out", [ROWS, D], f32, kind="ExternalOutput")

    with tile.TileContext(nc) as tc, contextlib.ExitStack() as ctx:
        const = ctx.enter_context(tc.tile_pool(name="const", bufs=1))
        wpool = ctx.enter_context(tc.tile_pool(name="wpool", bufs=1))
        act = ctx.enter_context(tc.tile_pool(name="act", bufs=1))
        xr_pool = ctx.enter_context(tc.tile_pool(name="xrp", bufs=6))
        n1_stat = ctx.enter_context(tc.tile_pool(name="n1_stat", bufs=4))

        ident_bf16 = const.tile([P, P], bf16)
        make_identity(nc, ident_bf16)
        ident_f32 = const.tile([P, P], f32)
        make_identity(nc, ident_f32)
        ones_row = const.tile([1, P], bf16)
        nc.vector.memset(ones_row, 1.0)
        eps_sb = const.tile([P, 1], f32)
        nc.vector.memset(eps_sb, EPS)

        # ---------------- DMA issue order --------------------------------
        # qACT: weights in first-use order.  qSP: x, small rows, ff tail.
        tT = const.tile([P, 4], bf16)
        nc.scalar.dma_start(tT, tT_d[:])
        ab = {}
        with (
            tc.tile_pool(name="nwp", bufs=1) as nwp,
            tc.tile_pool(name="embp", bufs=1) as embp,
        ):
            nw_sb = {}
            for l in (1, 2):
                nw_sb[l] = nwp.tile([P, 4, 2 * D], bf16, tag=f"nw{l}",
                                    name=f"nw_sb{l}")
                nc.scalar.dma_start(nw_sb[l], nw_d[l][:])

            a_sb = {}
            stacks = {}
            for a, wtag in ((1, "wbig1"), (2, "wbig2")):
                stack = wpool.tile([P, 4, 4, D], bf16, tag=wtag,
                                   name=f"a{a}stk")
                stacks[a] = stack
                for wi, w in enumerate("qkvo"):
                    a_sb[a, w] = stack[:, :, wi, :]
            pass
            for a in (1, 2):
                ob = wpool.tile([1, D], bf16, tag=f"a{a}ob", name=f"a{a}ob_sb")
                a_sb[a, "ob"] = ob

            h1T = act.tile([P, 4, NB], bf16, tag="tA")
            own_x = act.tile([P, 4, D], f32, tag="tE")
            x_tiles = {}
            for it in range(16):
                if it < 4:
                    dst = own_x[:, it, :]
                else:
                    dst = xr_pool.tile([P, D], f32, tag="xr", name=f"xr{it}")
                nc.gpsimd.dma_start(dst, xb[:][it * P:(it + 1) * P, :])
                x_tiles[it] = dst

            # condT + k2T share the tX region
            ctk2 = act.tile([P, 2, 4, NCTX], bf16, tag="tX")
            condT = ctk2[:, 0, :, :]
            k2T = ctk2[:, 1, :, :]
            # big weights + condT on the otherwise-idle SWDGE queue: HWDGE
            # trigger instructions cost ~2.5us EACH on their engine's queue
            # and were starving the ACT stats chain.
            nc.gpsimd.dma_start(stacks[1], as_d[1][:])
            nc.gpsimd.dma_start(a_sb[1, "ob"],
                                aob_d[1][:].rearrange("(a n) -> a n", a=1))
            nc.gpsimd.dma_start(stacks[2], as_d[2][:])
            nc.gpsimd.dma_start(a_sb[2, "ob"],
                                aob_d[2][:].rearrange("(a n) -> a n", a=1))
            nc.gpsimd.dma_start(condT, condT_d[:])
            nw_sb[4] = nwp.tile([P, 4, 2 * D], bf16, tag="nw1", name="nw_sb4")
            nc.gpsimd.dma_start(nw_sb[4], nw_d[4][:])

            nb_row = {}
            for l in (1, 2, 4):
                nb_row[l] = embp.tile([1, 2 * D], f32, tag="nbrow",
                                      name=f"nb_row{l}")
                nc.sync.dma_start(nb_row[l],
                                  nb_d[l][:].rearrange("(a n) -> a n", a=1))
            b1_sb = const.tile([P, 32], f32)
            nc.sync.dma_start(b1_sb, ffb1_d[:])
            b2_row = const.tile([1, D], bf16)
            nc.sync.dma_start(b2_row, ffb2_d[:].rearrange("(a n) -> a n", a=1))
            # ff weights on qSP after x: landed long before the FFN needs
            # them, but the buffers alias a1s/a2s so they wait on attention.
            w1_sb = wpool.tile([P, 4, 8 * D], bf16, tag="wbig1")
            nc.gpsimd.dma_start(w1_sb, ffw1_d[:])
            w2_sb = wpool.tile([P, 16, D], bf16, tag="wbig2")
            nc.gpsimd.dma_start(w2_sb, ffw2_d[:])

            # PE warmup: dependency-free matmuls fill the startup DMA window
            with tc.tile_pool(name="warm", bufs=1, space="PSUM") as warm_pool:
                wps = warm_pool.tile([P, P], f32)
                for _ in range(30):
                    nc.tensor.matmul(wps, ident_bf16, ident_bf16,
                                     start=True, stop=True)

            # ------------- norm scale/shift params -----------------------
            # only ab[1] gates the adaln1 loop; l=2,4 run after it so their
            # nw DMA arrival never stalls the PE stream.
            def emb_ab(l, ps_emb):
                emb_ps = ps_emb.tile([1, 2 * D], f32, tag="embps",
                                     name=f"emb_ps{l}")
                for half in range(2):
                    for kt in range(4):
                        nc.tensor.matmul(
                            emb_ps[:, half * D:(half + 1) * D],
                            tT[:, kt:kt + 1],
                            nw_sb[l][:, kt, half * D:(half + 1) * D],
                            start=(kt == 0), stop=(kt == 3),
                        )
                emb_row = embp.tile([1, 2 * D], f32, tag="embrow",
                                    name=f"emb_row{l}")
                nc.vector.tensor_tensor(emb_row, emb_ps, nb_row[l], op=OP.add)
                ab_l = const.tile([P, 8], f32, tag=f"ab{l}", name=f"ab_{l}")
                for col in range(8):
                    tp = ps_emb.tile([P, 1], f32, tag="embT")
                    nc.tensor.transpose(
                        tp, emb_row[0:1, col * P:(col + 1) * P],
                        ident_f32[0:1, 0:1]
                    )
                    nc.vector.tensor_scalar(
                        ab_l[:, col:col + 1], tp,
                        1.0 if col < 4 else 0.0, None, op0=OP.add,
                    )
                ab[l] = ab_l

            with tc.tile_pool(name="ps_emb1", bufs=2, space="PSUM") as pe1:
                emb_ab(1, pe1)

            # --------- adaln1 apply + projections, interleaved -----------
            # Per 512-row block: stats chunk -> xn (ACT) -> PE transpose ->
            # affine (DVE) -> dense k1/v1/q matmuls, with the PSUM->SBUF
            # drains on the otherwise-idle ACT engine.
            k1T = act.tile([P, 4, NB], bf16, tag="tB")
            v1 = act.tile([P, 16, H, HD + 1], bf16, tag="tC")
            q1pad = act.tile([P, 4, 2, ROWS], bf16, tag="tD")
            nc.gpsimd.memset(v1[:, :, :, HD:HD + 1], 1.0)
            nc.gpsimd.memset(q1pad, 0.0)
            mv1 = n1_stat.tile([P, 16, 2], f32)
            rstd1 = n1_stat.tile([P, 16], f32)
            nmr1 = n1_stat.tile([P, 16], f32)
            v2 = act.tile([P, 8, H, HD + 1], bf16, tag="tI")
            nc.gpsimd.memset(v2[:, :, :, HD:HD + 1], 1.0)
            with (
                tc.tile_pool(name="n1_xn", bufs=3) as xn_pool,
                tc.tile_pool(name="n1_pst", bufs=2, space="PSUM") as pst_pool,
                tc.tile_pool(name="ps_proj1", bufs=4, space="PSUM") as ps_proj,
            ):
                for jc in range(4):
                    defer = jc == 3   # jc3 chains ride as early attn1
                                      # fillers instead (needed from step ~9)
                    cs = slice(4 * jc, 4 * jc + 4)
                    for it in range(4 * jc, 4 * jc + 4):
                        stats = n1_stat.tile([P, 6], f32, tag="stats")
                        nc.vector.bn_stats(stats, x_tiles[it])
                        nc.vector.bn_aggr(mv1[:, it, :], stats)
                    nc.scalar.activation(rstd1[:, cs], mv1[:, cs, 1], AF.Sqrt,
                                         bias=eps_sb, scale=1.0)
                    nc.vector.reciprocal(rstd1[:, cs], rstd1[:, cs])
                    nc.vector.scalar_tensor_tensor(
                        nmr1[:, cs], mv1[:, cs, 0], -1.0, rstd1[:, cs],
                        op0=OP.mult, op1=OP.mult,
                    )
                    for it in range(4 * jc, 4 * jc + 4):
                        _adaln_apply_tile(nc, xn_pool, pst_pool, x_tiles[it],
                                          it, ab[1], rstd1, nmr1, h1T,
                                          ident_bf16, on_act=True)
                    for dt_ in range(4):
                        ps = ps_proj.tile([P, 512], f32, tag="proj")
                        for kt in range(4):
                            nc.tensor.matmul(
                                ps,
                                a_sb[1, "k"][:, kt, dt_ * P:(dt_ + 1) * P],
                                h1T[:, kt, jc * 512:(jc + 1) * 512],
                                start=(kt == 0), stop=(kt == 3),
                            )
                        nc.scalar.activation(
                            k1T[:, dt_, jc * 512:(jc + 1) * 512], ps, AF.Copy
                        )
                    for jt in range(4 * jc, 4 * jc + 4):
                        ps = ps_proj.tile([P, 512], f32, tag="proj")
                        for kt in range(4):
                            nc.tensor.matmul(
                                ps,
                                h1T[:, kt, jt * P:(jt + 1) * P],
                                a_sb[1, "v"][:, kt, :],
                                start=(kt == 0), stop=(kt == 3),
                            )
                        nc.scalar.activation(
                            v1[:, jt, :, 0:HD],
                            ps.rearrange("p (h d) -> p h d", h=H), AF.Copy
                        )
                    if jc == 0:
                        for dt_ in range(4):
                            ps = ps_proj.tile([P, 512], f32, tag="proj")
                            for kt in range(4):
                                nc.tensor.matmul(
                                    ps,
                                    a_sb[1, "q"][:, kt, dt_ * P:(dt_ + 1) * P],
                                    h1T[:, kt, 0:ROWS],
                                    start=(kt == 0), stop=(kt == 3),
                                )
                            nc.scalar.activation(q1pad[0:HD, dt_, 0, :],
                                                 ps[0:HD, :], AF.Copy)
                            nc.scalar.activation(q1pad[HD:P, dt_, 1, :],
                                                 ps[HD:P, :], AF.Copy)

            with tc.tile_pool(name="ps_emb2", bufs=2, space="PSUM") as pe2:
                emb_ab(2, pe2)
                emb_ab(4, pe2)

        # ---------------- attention 1 ------------------------------------
        # cross-attn K/V projections ride along as fillers in attn1's PE
        # slack (the exp stream is the bottleneck there); each chain uses
        # the one spare PSUM bank.
        def mk_k2(dt_, cjc):
            def f(pool):
                ps = pool.tile([P, 512], f32, tag="fps", name="fps")
                for kt in range(4):
                    nc.tensor.matmul(
                        ps,
                        a_sb[2, "k"][:, kt, dt_ * P:(dt_ + 1) * P],
                        condT[:, kt, cjc * 512:(cjc + 1) * 512],
                        start=(kt == 0), stop=(kt == 3),
                    )
                nc.vector.tensor_copy(
                    k2T[:, dt_, cjc * 512:(cjc + 1) * 512], ps
                )
            return f

        def mk_v2(jt):
            def f(pool):
                ps = pool.tile([P, 512], f32, tag="fps", name="fps")
                for kt in range(4):
                    nc.tensor.matmul(
                        ps,
                        condT[:, kt, jt * P:(jt + 1) * P],
                        a_sb[2, "v"][:, kt, :],
                        start=(kt == 0), stop=(kt == 3),
                    )
                nc.vector.tensor_copy(
                    v2[:, jt, :, 0:HD], ps.rearrange("p (h d) -> p h d", h=H)
                )
            return f

        def mk_k1jc3(dt_):
            def f(pool):
                ps = pool.tile([P, 512], f32, tag="fps", name="fps")
                for kt in range(4):
                    nc.tensor.matmul(
                        ps,
                        a_sb[1, "k"][:, kt, dt_ * P:(dt_ + 1) * P],
                        h1T[:, kt, 3 * 512:4 * 512],
                        start=(kt == 0), stop=(kt == 3),
                    )
                nc.vector.tensor_copy(k1T[:, dt_, 3 * 512:4 * 512], ps)
            return f

        def mk_v1(jt):
            def f(pool):
                ps = pool.tile([P, 512], f32, tag="fps", name="fps")
                for kt in range(4):
                    nc.tensor.matmul(
                        ps,
                        h1T[:, kt, jt * P:(jt + 1) * P],
                        a_sb[1, "v"][:, kt, :],
                        start=(kt == 0), stop=(kt == 3),
                    )
                nc.vector.tensor_copy(
                    v1[:, jt, :, 0:HD], ps.rearrange("p (h d) -> p h d", h=H)
                )
            return f

        fill1 = {}
        early = ([mk_k1jc3(0)] + [mk_v1(jt) for jt in range(12, 16)]
                 + [mk_k1jc3(1), mk_k1jc3(2), mk_k1jc3(3)])
        for i, fn in enumerate(early):
            fill1[i] = fn
        fns = ([mk_k2(dt_, cjc) for dt_ in range(4) for cjc in range(2)]
               + [mk_v2(jt) for jt in range(8)])
        for i, fn in enumerate(fns):
            fill1[20 + 2 * i] = fn

        x2 = act.tile([P, 4, D], f32, tag="tF")
        _attention(nc, tc, act, q1pad, k1T, v1, 16, a_sb[1, "o"],
                   a_sb[1, "ob"], ones_row, own_x, x2, "att1",
                   fillers=fill1)

        # ---------------- adaln2 + cross-attn ----------------------------
        h2T = act.tile([P, 4, ROWS], bf16, tag="tH")
        _adaln_to_hT(nc, tc, lambda it: x2[:, it, :], 4, ab[2], h2T,
                     ident_bf16, eps_sb, "n2")

        q2pad = act.tile([P, 4, 2, ROWS], bf16, tag="tD")
        nc.gpsimd.memset(q2pad, 0.0)
        with tc.tile_pool(name="ps_proj2b", bufs=2, space="PSUM") as ps_proj:
            for dt_ in range(4):
                ps = ps_proj.tile([P, 512], f32, tag="proj")
                for kt in range(4):
                    nc.tensor.matmul(
                        ps,
                        a_sb[2, "q"][:, kt, dt_ * P:(dt_ + 1) * P],
                        h2T[:, kt, :],
                        start=(kt == 0), stop=(kt == 3),
                    )
                nc.scalar.activation(q2pad[0:HD, dt_, 0, :], ps[0:HD, :],
                                     AF.Copy)
                nc.scalar.activation(q2pad[HD:P, dt_, 1, :], ps[HD:P, :],
                                     AF.Copy)

        x3 = act.tile([P, 4, D], f32, tag="tG")
        _attention(nc, tc, act, q2pad, k2T, v2, 8, a_sb[2, "o"],
                   a_sb[2, "ob"], ones_row, x2, x3, "att2")

        # ---------------- adaln3 + GEGLU FFN -----------------------------
        h3T = act.tile([P, 4, ROWS], bf16, tag="tJ")
        _adaln_to_hT(nc, tc, lambda it: x3[:, it, :], 4, ab[4], h3T,
                     ident_bf16, eps_sb, "n4")

        # per-ut pipeline: zu/zg -> gelu/stt -> 4 y-accumulator matmuls.
        # y accumulates in 4 persistent PSUM banks across all 16 ut chunks.
        ugT = act.tile([P, 16, ROWS], bf16, tag="tA")
        out_sb = act.tile([P, 4, D], f32, tag="tC")
        with (
            tc.tile_pool(name="ps_z", bufs=4, space="PSUM") as ps_z,
            tc.tile_pool(name="ps_y", bufs=1, space="PSUM") as ps_y,
            tc.tile_pool(name="gact", bufs=3) as gact_pool,
        ):
            y_ps = ps_y.tile([P, 4, D], f32)
            for ut in range(16):
                zu = ps_z.tile([P, ROWS], f32, tag="z")
                zg = ps_z.tile([P, ROWS], f32, tag="z")
                for kt in range(4):
                    nc.tensor.matmul(
                        zu, w1_sb[:, kt, ut * P:(ut + 1) * P],
                        h3T[:, kt, :], start=(kt == 0), stop=(kt == 3),
                    )
                for kt in range(4):
                    nc.tensor.matmul(
                        zg, w1_sb[:, kt, (16 + ut) * P:(17 + ut) * P],
                        h3T[:, kt, :], start=(kt == 0), stop=(kt == 3),
                    )
                gact = gact_pool.tile([P, ROWS], bf16, tag="gact")
                nc.scalar.activation(
                    gact, zg, AF.Gelu, bias=b1_sb[:, 16 + ut:17 + ut], scale=1.0
                )
                nc.vector.scalar_tensor_tensor(
                    ugT[:, ut, :], zu, b1_sb[:, ut:ut + 1], gact,
                    op0=OP.add, op1=OP.mult,
                )
                for it in range(4):
                    nc.tensor.matmul(
                        y_ps[:, it, :], ugT[:, ut, it * P:(it + 1) * P],
                        w2_sb[:, ut, :],
                        start=(ut == 0), stop=False,
                    )
            for it in range(4):
                nc.tensor.matmul(
                    y_ps[:, it, :], ones_row[0:1, 0:P], b2_row,
                    start=False, stop=True,
                )
                nc.vector.tensor_tensor(
                    out_sb[:, it, :], y_ps[:, it, :], x3[:, it, :], op=OP.add
                )
                nc.sync.dma_start(out[:][it * P:(it + 1) * P, :],
                                  out_sb[:, it, :])

    nc.compile()
    return nc


def _prep_shared(inputs):
    """Pre-cast weights to bf16 and pre-arrange into SBUF layouts (host-side
    layout prep, shared by all cores)."""
    bf = ml_dtypes.bfloat16

    def pkn(w, ktiles):
        # [ktiles*128, n] f32 -> [128, ktiles, n] bf16
        n = w.shape[1]
        return np.ascontiguousarray(
            w.reshape(ktiles, P, n).transpose(1, 0, 2).astype(bf))

    shared = {}
    for l in (1, 2, 4):
        shared[f"nw{l}"] = pkn(np.asarray(inputs[f"n{l}_w"], np.float32), 4)
        shared[f"nb{l}"] = np.ascontiguousarray(inputs[f"n{l}_b"], np.float32)
    for a in (1, 2):
        ws = [pkn(np.asarray(inputs[f"a{a}_{w}"], np.float32), 4)
              for w in "qkvo"]
        shared[f"a{a}s"] = np.ascontiguousarray(np.stack(ws, axis=2))
        shared[f"a{a}ob"] = np.asarray(inputs[f"a{a}_ob"], np.float32).astype(bf)
    shared["ffw1"] = pkn(np.asarray(inputs["ff_w1"], np.float32), 4)
    shared["ffw2"] = pkn(np.asarray(inputs["ff_w2"], np.float32), 16)
    shared["ffb1"] = np.ascontiguousarray(
        np.asarray(inputs["ff_b1"], np.float32).reshape(32, P).T)
    shared["ffb2"] = np.asarray(inputs["ff_b2"], np.float32).astype(bf)
    return shared


def _shard_inputs(inputs):
    """Build the 8 per-core input maps."""
    bf = ml_dtypes.bfloat16
    x = np.ascontiguousarray(inputs["x"], dtype=np.float32)
    t = np.ascontiguousarray(inputs["t"], dtype=np.float32)
    cond = np.ascontiguousarray(inputs["cond"], dtype=np.float32)
    shared = _prep_shared(inputs)
    per_batch = {}
    for b in range(B):
        condT = cond[b].T.reshape(4, P, NCTX).transpose(1, 0, 2)
        tT = t[b, 0].reshape(4, P).T
        per_batch[b] = (
            np.ascontiguousarray(condT.astype(bf)),
            np.ascontiguousarray(tT.astype(bf)),
        )
    in_maps = []
    for c in range(NCORES):
        b = c // 4
        r0 = (c % 4) * ROWS
        m = dict(shared)
        m["xb"] = np.ascontiguousarray(np.roll(x[b], -r0, axis=0))
        m["condT"], m["tT"] = per_batch[b]
        in_maps.append(m)
    return in_maps


def kernel(**inputs) -> np.ndarray:
    if "nc" not in _CACHED:
        _CACHED["nc"] = build()
    nc = _CACHED["nc"]
    in_maps = _shard_inputs(inputs)
    res = run_bass_kernel_spmd(nc, in_maps, core_ids=list(range(NCORES)))
    outs = [res.results[c]["out"] for c in range(NCORES)]
    full = np.concatenate(outs, axis=0).reshape(B, N, D)
    return full.astype(np.float32)
